# revision 17
# baseline (speedup 1.0000x reference)
"""Trainium2 Bass kernel for nn_Meta_67078799229377 (relation-network meta-learner).

Sharding: 8 cores = 4 batch elements x 2 halves of the relation-j axis.
Each core runs the full backbone for its batch element's 6 images, then the
relation network for its 18 (i, j) pairs, fully fused on-chip (the
[s,s,m,m,128] tensor never exists in HBM). Host code only reshapes/shards
inputs and combines 144 scores + 24 per-sample CE terms into the 3 scalar
losses.

v2 changes vs v1:
 - input DMAs reordered (patches first) and spread across engine queues
 - consolidated pad memsets, early ACT table prefetch (exp/ln)
 - hdd generation split across DVE/ACT/GPSIMD (env-tunable)
 - g evacuation in FD=2048 ops split ACT/DVE, PSUM tiles [128,2048]
 - score head uses two matmuls on xf partition halves (no SBUF-shift DMA)
 - cls output DMA issued as soon as ready
"""
import os
import numpy as np
import ml_dtypes

import concourse.bass as bass
import concourse.mybir as mybir
import concourse.tile as tile
from concourse import bacc
from concourse.bass_utils import run_bass_kernel_spmd

F32 = mybir.dt.float32
BF16 = mybir.dt.bfloat16
AF = mybir.ActivationFunctionType
OP = mybir.AluOpType


def _register_relu_bias_sum():
    """Custom DVE op: out = relu(in0 + s0), accum_out = sum(out).

    Fuses the g-evacuation (relu + bias + spatial-sum) into one Vector-engine
    instruction; stock tensor_scalar repurposes op1 as the reduce op when
    accum_out is attached, so it cannot express this.
    """
    from concourse import dve_ops
    from concourse.dve_spec import Spec, Src0, C0, Zero, relu, lower
    from concourse.dve_spec import _has_src1 as has_src1
    from concourse.dve_uop import DveOpSpec
    from operator import add as op_add

    name = "RELU_BIAS_SUM_ANT"
    for op in dve_ops.OPS:
        if op.name == name:
            return op

    def _ref(in0, in1, c0, c1, c2):
        b = np.maximum(in0.astype(np.float32) + c0, 0).astype(np.float32)
        return b, b.reshape(b.shape[0], -1).sum(axis=-1, keepdims=True)

    row = max(dve_ops._SUB_OPCODE_FOR_NAME.values()) + 1
    assert row < 0x20
    dve_ops._SUB_OPCODE_FOR_NAME[name] = row
    spec = Spec(body=relu(Src0 + C0), accum=op_add, accum_init=Zero,
                reference=_ref)
    shas = {}
    for ver in ("v3", "v4"):
        dspec = DveOpSpec(name=name, opcode=row, uops=lower(spec, ver=ver),
                          rd1_en=has_src1(spec))
        shas[ver] = dspec.sha(ver)
    op = dve_ops.DveOp(name, spec, subdim=False, uops_sha=shas)
    dve_ops.OPS.append(op)
    dve_ops.CUSTOM_DVE_SPECS[name] = spec
    return op


RELU_BIAS_SUM = None
if int(os.environ.get("KDVE_EVAC", "0")) > 0:
    RELU_BIAS_SUM = _register_relu_bias_sum()

B, S, D = 4, 6, 8
M = D * D            # 64 spatial positions
C2 = 66              # 64 channels + 2 coord channels
H1 = 128             # g-MLP hidden
CO = 64              # g-MLP out
NCls = 64
N_CORES = 8

# hdd-gen engine split per unit (32 q-ops): gpsimd + act counts; rest on DVE.
KGPS = int(os.environ.get("KGPS", "0"))
KACT = int(os.environ.get("KACT", "2"))
# of the 6 per-unit evacuation ops, how many go to DVE (rest on ACT)
KDVE_EVAC = int(os.environ.get("KDVE_EVAC", "0"))
# which engines take non-sync const DMAs: "sync" | "spread"
KDMA = os.environ.get("KDMA", "sync")


def _build_nc():
    nc = bacc.Bacc("TRN2", target_bir_lowering=False, debug=False,
                   num_devices=N_CORES)

    din = {}
    def dram_in(name, shape, dtype=F32):
        din[name] = nc.dram_tensor(name, list(shape), dtype, kind="ExternalInput")
        return din[name]

    x_patches = dram_in("patches", [27, S, 1024], BF16)
    x_w1 = dram_in("w1", [27, 32], BF16)
    x_w2 = dram_in("w2", [32, 9 * 48], BF16)
    x_w3 = dram_in("w3", [48, 9 * 64], BF16)
    x_bc1 = dram_in("bc1", [32, 1])
    x_bc2 = dram_in("bc2", [48, 1])
    x_bc3 = dram_in("bc3", [64, 1])
    x_coords = dram_in("coords", [2, S * M], BF16)
    x_wle = dram_in("wle", [65, NCls])
    x_onehot = dram_in("onehot", [S, NCls])
    x_w1a = dram_in("w1a", [C2, H1], BF16)
    x_w1b = dram_in("w1b", [C2, H1], BF16)
    x_bg1 = dram_in("bg1", [H1, 1])
    x_wg2 = dram_in("wg2", [H1, CO], BF16)
    x_bg2 = dram_in("bg2_2", [2 * CO, 1])
    x_wf1d = dram_in("wf1d", [2 * CO, 16])   # Wf1 stacked twice (for row halves)
    x_bf1 = dram_in("bf1", [16, 1])
    x_wf2e = dram_in("wf2e", [17, 1])

    out_scores = nc.dram_tensor("scores", [18, 1], F32, kind="ExternalOutput")
    out_cls = nc.dram_tensor("clsv", [S, 1], F32, kind="ExternalOutput")

    with tile.TileContext(nc) as tc:
        with (
            tc.tile_pool(name="const", bufs=1) as cpool,
            tc.tile_pool(name="work", bufs=1) as wpool,
            tc.tile_pool(name="patch", bufs=1) as ppool,
            tc.tile_pool(name="hdd", bufs=3) as hpool,
            tc.tile_pool(name="gscr", bufs=2) as spool,
            tc.tile_pool(name="psmall", bufs=2, space="PSUM") as pc_pool,
            tc.tile_pool(name="pbig", bufs=2, space="PSUM") as pb_pool,
        ):
            # ---- inputs to SBUF; order matters: conv-critical first on sync ----
            def c_tile(src, shape, dtype=F32, eng=None):
                t = cpool.tile(list(shape), dtype, tag=src.name)
                (eng or nc.sync).dma_start(out=t[:], in_=src[:])
                return t

            patches_sb = ppool.tile([27, S, 1024], BF16)
            nc.sync.dma_start(out=patches_sb[:], in_=x_patches[:])
            w1_sb = c_tile(x_w1, [27, 32], BF16)
            bc1_sb = c_tile(x_bc1, [32, 1])
            w2_sb = c_tile(x_w2, [32, 9 * 48], BF16)
            bc2_sb = c_tile(x_bc2, [48, 1])
            w3_sb = c_tile(x_w3, [48, 9 * 64], BF16)
            bc3_sb = c_tile(x_bc3, [64, 1])

            featc = wpool.tile([C2, S * M], BF16)
            # non-critical consts; optionally spread across the scalar queue
            # (only SP/Activation/gpsimd can initiate DMAs)
            alt = nc.scalar if KDMA == "spread" else nc.sync
            nc.sync.dma_start(out=featc[64:66, :], in_=x_coords[:])
            w1a_sb = c_tile(x_w1a, [C2, H1], BF16, eng=alt)
            w1b_sb = c_tile(x_w1b, [C2, H1], BF16, eng=alt)
            bg1_sb = c_tile(x_bg1, [H1, 1], eng=alt)
            wg2_sb = c_tile(x_wg2, [H1, CO], BF16, eng=alt)
            bg2_sb = c_tile(x_bg2, [2 * CO, 1], eng=alt)
            wle_sb = c_tile(x_wle, [65, NCls])
            onehot_sb = c_tile(x_onehot, [S, NCls])
            wf1d_sb = c_tile(x_wf1d, [2 * CO, 16])
            bf1_sb = c_tile(x_bf1, [16, 1])
            wf2e_sb = c_tile(x_wf2e, [17, 1])

            def r32(ap):
                return ap

            # ---- ACT table prefetch: force exp/ln tables to load during the
            # conv phase instead of stalling the score head at the end ----
            tiny = wpool.tile([64, 4], F32, tag="tiny")
            nc.gpsimd.memset(tiny[:], 1.0)
            nc.scalar.activation(tiny[:, 2:3], tiny[:, 0:1], AF.Exp)
            nc.scalar.activation(tiny[:, 3:4], tiny[:, 1:2], AF.Ln)

            # ---- conv1: [27]->[32], 64x64 -> 32x32 (stride 2, im2col'd) ----
            c1sb = wpool.tile([32, S, 33, 33], BF16)
            c2sb = wpool.tile([48, S, 17, 17], BF16)
            for img in range(S):
                nc.gpsimd.memset(c1sb[:, img, 32, :], 0.0)
                nc.gpsimd.memset(c1sb[:, img, 0:32, 32], 0.0)
                nc.gpsimd.memset(c2sb[:, img, 16, :], 0.0)
                nc.gpsimd.memset(c2sb[:, img, 0:16, 16], 0.0)

            for img in range(S):
                for h in range(2):
                    ps1 = pc_pool.tile([32, 16, 32], F32, tag="psc")
                    nc.tensor.matmul(
                        ps1[:].rearrange("p a b -> p (a b)"),
                        r32(w1_sb[:]),
                        r32(patches_sb[:, img, h * 512:(h + 1) * 512]),
                        start=True, stop=True)
                    # relu(x + bc1) -> padded layout; DVE (ACT is busier later)
                    out_ap = c1sb[:, img, h * 16:(h + 1) * 16, 0:32]
                    if img % 3 == 2:
                        nc.scalar.activation(out_ap, ps1[:], AF.Relu, bias=bc1_sb[:])
                    else:
                        nc.vector.tensor_scalar(out_ap, ps1[:], bc1_sb[:], 0.0,
                                                op0=OP.add, op1=OP.max)

            # ---- conv2: [32]->[48], 32x32 -> 16x16 ----
            for ip in range(3):      # image pairs
                ps2 = pc_pool.tile([48, 2, 16, 16], F32, tag="psc")
                for k, (dy, dx) in enumerate((dy, dx) for dy in range(3) for dx in range(3)):
                    nc.tensor.matmul(
                        ps2[:],
                        r32(w2_sb[:, k * 48:(k + 1) * 48]),
                        r32(c1sb[:, 2 * ip:2 * ip + 2, dy:dy + 31:2, dx:dx + 31:2]),
                        start=(k == 0), stop=(k == 8))
                out_ap = c2sb[:, 2 * ip:2 * ip + 2, 0:16, 0:16]
                if ip % 2 == 0:
                    nc.scalar.activation(out_ap, ps2[:], AF.Relu, bias=bc2_sb[:])
                else:
                    nc.vector.tensor_scalar(out_ap, ps2[:], bc2_sb[:], 0.0,
                                            op0=OP.add, op1=OP.max)

            # ---- conv3: [48]->[64], 16x16 -> 8x8 ----
            ps3 = pc_pool.tile([64, S, D, D], F32, tag="psc")
            for k, (dy, dx) in enumerate((dy, dx) for dy in range(3) for dx in range(3)):
                nc.tensor.matmul(
                    ps3[:],
                    r32(w3_sb[:, k * 64:(k + 1) * 64]),
                    r32(c2sb[:, :, dy:dy + 15:2, dx:dx + 15:2]),
                    start=(k == 0), stop=(k == 8))
            nc.scalar.activation(featc[0:64, :].rearrange("p (i m) -> p i m", m=M),
                                 ps3[:].rearrange("p i a b -> p i (a b)"),
                                 AF.Relu, bias=bc3_sb[:])

            # ---- u / v ----
            psu = pc_pool.tile([H1, S * M], F32, tag="psc")
            psv = pc_pool.tile([H1, S * M], F32, tag="psc")
            nc.tensor.matmul(psu[:], r32(w1a_sb[:]), r32(featc[:]), start=True, stop=True)
            nc.tensor.matmul(psv[:], r32(w1b_sb[:]), r32(featc[:]), start=True, stop=True)
            u_f32 = wpool.tile([H1, S * M], F32)
            v_bf = wpool.tile([H1, S * M], BF16)
            v_f32 = wpool.tile([H1, S * M], F32)
            nc.scalar.activation(u_f32[:], psu[:], AF.Copy)
            nc.vector.tensor_scalar(v_bf[:], psv[:], bg1_sb[:], None, op0=OP.add)
            nc.vector.tensor_scalar(v_f32[:], psv[:], bg1_sb[:], None, op0=OP.add)

            # ---- cls head (overlaps relation; DMA result out early) ----
            fme = wpool.tile([65, S], F32)
            nc.gpsimd.memset(fme[:], 1.0)
            nc.vector.tensor_reduce(
                fme[0:64, :], featc[0:64, :].rearrange("p (i m) -> p i m", m=M),
                axis=mybir.AxisListType.X, op=OP.add)
            psl = pc_pool.tile([S, NCls], F32, tag="psc")
            nc.tensor.matmul(psl[:], r32(fme[:]), r32(wle_sb[:]), start=True, stop=True)
            mx = wpool.tile([S, 1], F32)
            nc.vector.tensor_reduce(mx[:], psl[:], axis=mybir.AxisListType.X, op=OP.max)
            shifted = wpool.tile([S, NCls], F32)
            nc.vector.tensor_scalar(shifted[:], psl[:], mx[:], None, op0=OP.subtract)
            escr = wpool.tile([S, NCls], F32)
            se = wpool.tile([S, 1], F32)
            nc.scalar.activation(escr[:], shifted[:], AF.Exp, accum_out=se[:])
            lse = wpool.tile([S, 1], F32)
            nc.scalar.activation(lse[:], se[:], AF.Ln)
            selscr = wpool.tile([S, NCls], F32)
            sel = wpool.tile([S, 1], F32)
            nc.vector.tensor_tensor(selscr[:], shifted[:], onehot_sb[:], op=OP.mult)
            nc.vector.tensor_reduce(sel[:], selscr[:], axis=mybir.AxisListType.X, op=OP.add)
            clsv = wpool.tile([S, 1], F32)
            nc.vector.tensor_tensor(clsv[:], lse[:], sel[:], op=OP.subtract)
            nc.sync.dma_start(out=out_cls[:], in_=clsv[:])

            # ---- relation stage ----
            # xf_cols[:, (qh*9 + jl*3 + duo) + 18*gh] accumulates one evac op's
            # sum over (16q x 64p); combine gh then qh afterwards.
            xf_cols = wpool.tile([2 * CO, 36], F32)

            # engine assignment pattern for the 32 hdd-gen q ops of each unit
            gps_slots = set(int(i * 32 / KGPS) for i in range(KGPS)) if KGPS else set()
            rest = [s for s in range(32) if s not in gps_slots]
            act_slots = set(rest[int((i + 0.5) * len(rest) / KACT)]
                            for i in range(KACT)) if KACT else set()
            dve_evac_slots = (set(int((i + 0.5) * 6 / KDVE_EVAC)
                                  for i in range(KDVE_EVAC))
                              if KDVE_EVAC else set())

            for jl in range(3):
                for qh in range(2):
                    hdd = hpool.tile([H1, 32, S * M], BF16, tag="hdd")
                    for ql in range(32):
                        q = qh * 32 + ql
                        ucol = u_f32[:, jl * M + q: jl * M + q + 1]
                        if ql in act_slots:
                            nc.scalar.activation(hdd[:, ql, :], v_f32[:],
                                                 AF.Relu, bias=ucol)
                        elif ql in gps_slots:
                            nc.gpsimd.tensor_scalar(hdd[:, ql, :], v_f32[:],
                                                    ucol, 0.0,
                                                    op0=OP.add, op1=OP.max)
                        else:
                            nc.vector.tensor_scalar(hdd[:, ql, :], v_bf[:],
                                                    ucol, 0.0,
                                                    op0=OP.add, op1=OP.max)
                    for duo in range(3):
                        iA, iB = 2 * duo, 2 * duo + 1
                        for gh in range(2):
                            ps = pb_pool.tile([2 * CO, 1024], F32, tag="gps")
                            for q2 in range(2):
                                qg = gh * 2 + q2
                                nc.tensor.matmul(
                                    ps[0:CO, q2 * 512:(q2 + 1) * 512],
                                    wg2_sb[:],
                                    hdd[:, qg * 8:(qg + 1) * 8, iA * M:(iA + 1) * M],
                                    start=True, stop=True)
                                nc.tensor.matmul(
                                    ps[CO:2 * CO, q2 * 512:(q2 + 1) * 512],
                                    wg2_sb[:],
                                    hdd[:, qg * 8:(qg + 1) * 8, iB * M:(iB + 1) * M],
                                    start=True, stop=True,
                                    tile_position=(0, 64))
                            ucol_i = (qh * 9 + jl * 3 + duo) + 18 * gh
                            gscr = spool.tile([2 * CO, 1024], BF16, tag="gscr")
                            if (duo * 2 + gh) in dve_evac_slots:
                                nc.vector._custom_dve(
                                    RELU_BIAS_SUM, out=gscr[:], in0=ps[:],
                                    s0=bg2_sb[:],
                                    accum_out=xf_cols[:, ucol_i:ucol_i + 1])
                            else:
                                nc.scalar.activation(
                                    gscr[:], ps[:], AF.Relu,
                                    bias=bg2_sb[:],
                                    accum_out=xf_cols[:, ucol_i:ucol_i + 1])

            # ---- score head ----
            # sum the two gh-halves, then the two qh-halves
            xf18 = wpool.tile([2 * CO, 18], F32)
            nc.vector.tensor_tensor(
                xf18[:], xf_cols[:, 0:18], xf_cols[:, 18:36], op=OP.add)
            xf_pair = wpool.tile([2 * CO, 9], F32)
            nc.vector.tensor_tensor(
                xf_pair[:], xf18[:, 0:9], xf18[:, 9:18], op=OP.add)
            # assemble xf for both partition halves into one base-0 tile:
            # even i (rows 0:64) -> cols 0:9, odd i -> cols 9:18 (SBUF DMA)
            xf_ext = wpool.tile([CO, 18], F32)
            nc.vector.tensor_copy(xf_ext[:, 0:9], xf_pair[0:CO, :])
            nc.sync.dma_start(out=xf_ext[:, 9:18], in_=xf_pair[CO:2 * CO, :])
            psh1 = pc_pool.tile([16, 18], F32, tag="psc")
            nc.tensor.matmul(psh1[:], r32(wf1d_sb[0:CO, :]),
                             r32(xf_ext[:]), start=True, stop=True)
            h1e = wpool.tile([17, 18], F32)
            nc.gpsimd.memset(h1e[:], 1.0)
            nc.scalar.activation(h1e[0:16, :], psh1[:], AF.Relu, bias=bf1_sb[:])
            psh2 = pc_pool.tile([18, 1], F32, tag="psc")
            nc.tensor.matmul(psh2[:], r32(h1e[:]), r32(wf2e_sb[:]),
                             start=True, stop=True)
            en = wpool.tile([18, 1], F32)
            nc.scalar.activation(en[:], psh2[:], AF.Exp, scale=-1.0)
            ep1 = wpool.tile([18, 1], F32)
            nc.vector.tensor_scalar(ep1[:], en[:], 1.0, None, op0=OP.add)
            sc = wpool.tile([18, 1], F32)
            nc.vector.reciprocal(sc[:], ep1[:])
            nc.sync.dma_start(out=out_scores[:], in_=sc[:])
    nc.compile()
    return nc


_NC_CACHE = None


def _get_nc():
    global _NC_CACHE
    if _NC_CACHE is None:
        _NC_CACHE = _build_nc()
    return _NC_CACHE


def _host_prep(inputs):
    ins = {k: np.asarray(v) for k, v in inputs.items()}
    x = np.concatenate([ins['support_x'], ins['query_x']], axis=1)
    lab = np.concatenate([ins['support_y'], ins['query_y']], axis=1)

    xpad = np.pad(x.astype(np.float32), ((0, 0), (0, 0), (0, 0), (0, 1), (0, 1)))
    win = np.lib.stride_tricks.sliding_window_view(xpad, (3, 3), axis=(3, 4))
    win = win[:, :, :, ::2, ::2]
    patches = win.transpose(0, 2, 5, 6, 1, 3, 4).reshape(B, 27, S, 1024)
    patches = np.ascontiguousarray(patches, np.float32)

    f32 = np.float32
    bf16 = ml_dtypes.bfloat16
    w1 = np.ascontiguousarray(ins['k1'].reshape(32, 27).T, f32).astype(bf16)
    w2 = np.ascontiguousarray(ins['k2'].transpose(1, 2, 3, 0).reshape(32, 9 * 48), f32).astype(bf16)
    w3 = np.ascontiguousarray(ins['k3'].transpose(1, 2, 3, 0).reshape(48, 9 * 64), f32).astype(bf16)

    ii = np.arange(D, dtype=f32) / D
    coord = np.stack([np.broadcast_to(ii[:, None], (D, D)),
                      np.broadcast_to(ii[None, :], (D, D))]).reshape(2, M)
    coords = np.ascontiguousarray(np.tile(coord, (1, S)), f32).astype(bf16)

    onehots = np.zeros((B, S, NCls), f32)
    for b in range(B):
        onehots[b, np.arange(S), lab[b]] = 1.0

    Wg1 = ins['Wg1'].astype(f32)
    common = dict(
        w1=w1, w2=w2, w3=w3,
        bc1=np.ascontiguousarray(ins['bc1'].reshape(32, 1), f32),
        bc2=np.ascontiguousarray(ins['bc2'].reshape(48, 1), f32),
        bc3=np.ascontiguousarray(ins['bc3'].reshape(64, 1), f32),
        coords=coords,
        wle=np.ascontiguousarray(
            np.vstack([ins['Wlog'].astype(f32) / M, ins['blog'][None, :].astype(f32)])),
        w1a=np.ascontiguousarray(Wg1[:C2]).astype(bf16),
        w1b=np.ascontiguousarray(Wg1[C2:]).astype(bf16),
        bg1=np.ascontiguousarray(ins['bg1'].reshape(H1, 1), f32),
        wg2=np.ascontiguousarray(ins['Wg2'], f32).astype(ml_dtypes.bfloat16),
        bg2_2=np.ascontiguousarray(np.tile(ins['bg2'].astype(f32), 2).reshape(2 * CO, 1)),
        wf1d=np.ascontiguousarray(
            np.vstack([ins['Wf1'].astype(f32), ins['Wf1'].astype(f32)])),
        bf1=np.ascontiguousarray(ins['bf1'].reshape(16, 1), f32),
        wf2e=np.ascontiguousarray(
            np.vstack([ins['Wf2'].astype(f32), ins['bf2'].reshape(1, 1).astype(f32)])),
    )
    in_maps = []
    for core in range(N_CORES):
        b, half = core // 2, core % 2
        # odd cores see images in rotated order so the program's local
        # j in {0,1,2} maps to global j in {3,4,5}
        perm = (0, 1, 2, 3, 4, 5) if half == 0 else (3, 4, 5, 0, 1, 2)
        m = dict(common)
        m['patches'] = np.ascontiguousarray(patches[b][:, perm, :]).astype(ml_dtypes.bfloat16)
        m['onehot'] = np.ascontiguousarray(onehots[b][list(perm)])
        in_maps.append(m)
    return in_maps, lab


def _host_post(results, lab):
    P = np.zeros((B, S, S), np.float32)
    cls_terms = np.zeros((B, S), np.float32)
    for core in range(N_CORES):
        b, half = core // 2, core % 2
        perm = (0, 1, 2, 3, 4, 5) if half == 0 else (3, 4, 5, 0, 1, 2)
        sc = results[core]["scores"].reshape(18)
        # score col layout: k < 9: (jl = k//3, duo = k%3, i = 2*duo);
        #                   k >= 9: i = 2*duo + 1
        for k in range(18):
            kk = k % 9
            jl, duo = kk // 3, kk % 3
            i = 2 * duo + (1 if k >= 9 else 0)
            P[b, perm[i], perm[jl]] = sc[k]
        if half == 0:
            cls_terms[b] = results[core]["clsv"].reshape(S)
    cls_loss = np.float32(cls_terms.mean())
    y = (lab[:, :, None] == lab[:, None, :]).astype(np.float32)
    Pt = P.transpose(0, 2, 1)
    sym, anti = np.float32(0.5) * (P + Pt), np.float32(0.5) * (P - Pt)
    sym_n = np.sqrt((sym ** 2).sum(axis=(1, 2)))
    anti_n = np.sqrt((anti ** 2).sum(axis=(1, 2)))
    sym_loss = np.float32(((sym_n - anti_n) / (sym_n + anti_n)).mean())
    euc_loss = np.float32(((P - y) ** 2).mean())
    rn_loss = np.float32(euc_loss - np.float32(0.1) * sym_loss)
    return np.float32(cls_loss), np.float32(rn_loss), np.float32(sym_loss)


def run_spmd(inputs, trace=False, **kwargs):
    nc = _get_nc()
    in_maps, lab = _host_prep(inputs)
    res = run_bass_kernel_spmd(nc, in_maps, list(range(N_CORES)),
                               trace=trace, **kwargs)
    return _host_post(res.results, lab), res


def kernel(**inputs):
    out, _ = run_spmd(inputs)
    return out


# revision 19
# speedup vs baseline: 3.7144x; 3.7144x over previous
"""Trainium2 Bass kernel for nn_Meta_67078799229377 (relation-network meta-learner).

Sharding: 8 cores = 4 batch elements x 2 halves of the relation-j axis.
Each core runs the full backbone for its batch element's 6 images, then the
relation network for its 18 (i, j) pairs, fully fused on-chip (the
[s,s,m,m,128] tensor never exists in HBM). Host code only reshapes/shards
inputs and combines 144 scores + 24 per-sample CE terms into the 3 scalar
losses.

v2 changes vs v1:
 - input DMAs reordered (patches first) and spread across engine queues
 - consolidated pad memsets, early ACT table prefetch (exp/ln)
 - hdd generation split across DVE/ACT/GPSIMD (env-tunable)
 - g evacuation in FD=2048 ops split ACT/DVE, PSUM tiles [128,2048]
 - score head uses two matmuls on xf partition halves (no SBUF-shift DMA)
 - cls output DMA issued as soon as ready
"""
import os
import numpy as np
import ml_dtypes

import concourse.bass as bass
import concourse.mybir as mybir
import concourse.tile as tile
from concourse import bacc
from concourse.bass_utils import run_bass_kernel_spmd

F32 = mybir.dt.float32
BF16 = mybir.dt.bfloat16
AF = mybir.ActivationFunctionType
OP = mybir.AluOpType


def _register_relu_bias_sum():
    """Custom DVE op: out = relu(in0 + s0), accum_out = sum(out).

    Fuses the g-evacuation (relu + bias + spatial-sum) into one Vector-engine
    instruction; stock tensor_scalar repurposes op1 as the reduce op when
    accum_out is attached, so it cannot express this.
    """
    from concourse import dve_ops
    from concourse.dve_spec import Spec, Src0, C0, Zero, relu, lower
    from concourse.dve_spec import _has_src1 as has_src1
    from concourse.dve_uop import DveOpSpec
    from operator import add as op_add

    name = "RELU_BIAS_SUM_ANT"
    for op in dve_ops.OPS:
        if op.name == name:
            return op

    def _ref(in0, in1, c0, c1, c2):
        b = np.maximum(in0.astype(np.float32) + c0, 0).astype(np.float32)
        return b, b.reshape(b.shape[0], -1).sum(axis=-1, keepdims=True)

    row = max(dve_ops._SUB_OPCODE_FOR_NAME.values()) + 1
    assert row < 0x20
    dve_ops._SUB_OPCODE_FOR_NAME[name] = row
    spec = Spec(body=relu(Src0 + C0), accum=op_add, accum_init=Zero,
                reference=_ref)
    shas = {}
    for ver in ("v3", "v4"):
        dspec = DveOpSpec(name=name, opcode=row, uops=lower(spec, ver=ver),
                          rd1_en=has_src1(spec))
        shas[ver] = dspec.sha(ver)
    op = dve_ops.DveOp(name, spec, subdim=False, uops_sha=shas)
    dve_ops.OPS.append(op)
    dve_ops.CUSTOM_DVE_SPECS[name] = spec
    return op


RELU_BIAS_SUM = None
if int(os.environ.get("KDVE_EVAC", "0")) > 0:
    RELU_BIAS_SUM = _register_relu_bias_sum()

B, S, D = 4, 6, 8
M = D * D            # 64 spatial positions
C2 = 66              # 64 channels + 2 coord channels
H1 = 128             # g-MLP hidden
CO = 64              # g-MLP out
NCls = 64
N_CORES = 8

# hdd-gen engine split per unit (32 q-ops): gpsimd + act counts; rest on DVE.
KGPS = int(os.environ.get("KGPS", "0"))
KACT = int(os.environ.get("KACT", "2"))
# of the 6 per-unit evacuation ops, how many go to DVE (rest on ACT)
KDVE_EVAC = int(os.environ.get("KDVE_EVAC", "0"))
# which engines take non-sync const DMAs: "sync" | "spread"
KDMA = os.environ.get("KDMA", "sync")


def _build_nc():
    nc = bacc.Bacc("TRN2", target_bir_lowering=False, debug=False,
                   num_devices=N_CORES)

    din = {}
    def dram_in(name, shape, dtype=F32):
        din[name] = nc.dram_tensor(name, list(shape), dtype, kind="ExternalInput")
        return din[name]

    x_patches = dram_in("patches", [27, S, 1024], BF16)
    x_w1 = dram_in("w1", [27, 32], BF16)
    x_w2 = dram_in("w2", [32, 9 * 48], BF16)
    x_w3 = dram_in("w3", [48, 9 * 64], BF16)
    x_bc1 = dram_in("bc1", [32, 1])
    x_bc2 = dram_in("bc2", [48, 1])
    x_bc3 = dram_in("bc3", [64, 1])
    x_coords = dram_in("coords", [2, S * M], BF16)
    x_wle = dram_in("wle", [65, NCls])
    x_onehot = dram_in("onehot", [S, NCls])
    x_w1a = dram_in("w1a", [C2, H1], BF16)
    x_w1b = dram_in("w1b", [C2, H1], BF16)
    x_bg1 = dram_in("bg1", [H1, 1])
    x_wg2 = dram_in("wg2", [H1, CO], BF16)
    x_bg2 = dram_in("bg2_2", [2 * CO, 1])
    x_wf1d = dram_in("wf1d", [2 * CO, 16])   # Wf1 stacked twice (for row halves)
    x_bf1 = dram_in("bf1", [16, 1])
    x_wf2e = dram_in("wf2e", [17, 1])

    out_scores = nc.dram_tensor("scores", [18, 1], F32, kind="ExternalOutput")
    out_cls = nc.dram_tensor("clsv", [S, 1], F32, kind="ExternalOutput")

    with tile.TileContext(nc) as tc:
        with (
            tc.tile_pool(name="const", bufs=1) as cpool,
            tc.tile_pool(name="work", bufs=1) as wpool,
            tc.tile_pool(name="patch", bufs=1) as ppool,
            tc.tile_pool(name="hdd", bufs=3) as hpool,
            tc.tile_pool(name="gscr", bufs=2) as spool,
            tc.tile_pool(name="psmall", bufs=1, space="PSUM") as pc_pool,
            tc.tile_pool(name="pbig", bufs=3, space="PSUM") as pb_pool,
        ):
            # ---- inputs to SBUF; order matters: conv-critical first on sync ----
            def c_tile(src, shape, dtype=F32, eng=None):
                t = cpool.tile(list(shape), dtype, tag=src.name)
                (eng or nc.sync).dma_start(out=t[:], in_=src[:])
                return t

            patches_sb = ppool.tile([27, S, 1024], BF16)
            nc.sync.dma_start(out=patches_sb[:], in_=x_patches[:])
            w1_sb = c_tile(x_w1, [27, 32], BF16)
            bc1_sb = c_tile(x_bc1, [32, 1])
            w2_sb = c_tile(x_w2, [32, 9 * 48], BF16)
            bc2_sb = c_tile(x_bc2, [48, 1])
            w3_sb = c_tile(x_w3, [48, 9 * 64], BF16)
            bc3_sb = c_tile(x_bc3, [64, 1])

            featc = wpool.tile([C2, S * M], BF16)
            # non-critical consts; optionally spread across the scalar queue
            # (only SP/Activation/gpsimd can initiate DMAs)
            alt = nc.scalar if KDMA == "spread" else nc.sync
            nc.sync.dma_start(out=featc[64:66, :], in_=x_coords[:])
            w1a_sb = c_tile(x_w1a, [C2, H1], BF16, eng=alt)
            w1b_sb = c_tile(x_w1b, [C2, H1], BF16, eng=alt)
            bg1_sb = c_tile(x_bg1, [H1, 1], eng=alt)
            wg2_sb = c_tile(x_wg2, [H1, CO], BF16, eng=alt)
            bg2_sb = c_tile(x_bg2, [2 * CO, 1], eng=alt)
            wle_sb = c_tile(x_wle, [65, NCls])
            onehot_sb = c_tile(x_onehot, [S, NCls])
            wf1d_sb = c_tile(x_wf1d, [2 * CO, 16])
            bf1_sb = c_tile(x_bf1, [16, 1])
            wf2e_sb = c_tile(x_wf2e, [17, 1])

            def r32(ap):
                return ap

            # ---- conv1: [27]->[32], 64x64 -> 32x32 (stride 2, im2col'd) ----
            c1sb = wpool.tile([32, S, 33, 33], BF16)
            c2sb = wpool.tile([48, S, 17, 17], BF16)
            for img in range(S):
                nc.gpsimd.memset(c1sb[:, img, 32, :], 0.0)
                nc.gpsimd.memset(c1sb[:, img, 0:32, 32], 0.0)
                nc.gpsimd.memset(c2sb[:, img, 16, :], 0.0)
                nc.gpsimd.memset(c2sb[:, img, 0:16, 16], 0.0)

            for img in range(S):
                for h in range(2):
                    ps1 = pc_pool.tile([32, 16, 32], F32, tag="psc")
                    nc.tensor.matmul(
                        ps1[:].rearrange("p a b -> p (a b)"),
                        r32(w1_sb[:]),
                        r32(patches_sb[:, img, h * 512:(h + 1) * 512]),
                        start=True, stop=True)
                    # relu(x + bc1) -> padded layout; DVE (ACT is busier later)
                    out_ap = c1sb[:, img, h * 16:(h + 1) * 16, 0:32]
                    if img % 3 == 2:
                        nc.scalar.activation(out_ap, ps1[:], AF.Relu, bias=bc1_sb[:])
                    else:
                        nc.vector.tensor_scalar(out_ap, ps1[:], bc1_sb[:], 0.0,
                                                op0=OP.add, op1=OP.max)

            # ---- conv2: [32]->[48], 32x32 -> 16x16 ----
            for ip in range(3):      # image pairs
                ps2 = pc_pool.tile([48, 2, 16, 16], F32, tag="psc")
                for k, (dy, dx) in enumerate((dy, dx) for dy in range(3) for dx in range(3)):
                    nc.tensor.matmul(
                        ps2[:],
                        r32(w2_sb[:, k * 48:(k + 1) * 48]),
                        r32(c1sb[:, 2 * ip:2 * ip + 2, dy:dy + 31:2, dx:dx + 31:2]),
                        start=(k == 0), stop=(k == 8))
                out_ap = c2sb[:, 2 * ip:2 * ip + 2, 0:16, 0:16]
                if ip % 2 == 0:
                    nc.scalar.activation(out_ap, ps2[:], AF.Relu, bias=bc2_sb[:])
                else:
                    nc.vector.tensor_scalar(out_ap, ps2[:], bc2_sb[:], 0.0,
                                            op0=OP.add, op1=OP.max)

            # ---- conv3: [48]->[64], 16x16 -> 8x8 ----
            ps3 = pc_pool.tile([64, S, D, D], F32, tag="psc")
            for k, (dy, dx) in enumerate((dy, dx) for dy in range(3) for dx in range(3)):
                nc.tensor.matmul(
                    ps3[:],
                    r32(w3_sb[:, k * 64:(k + 1) * 64]),
                    r32(c2sb[:, :, dy:dy + 15:2, dx:dx + 15:2]),
                    start=(k == 0), stop=(k == 8))
            nc.scalar.activation(featc[0:64, :].rearrange("p (i m) -> p i m", m=M),
                                 ps3[:].rearrange("p i a b -> p i (a b)"),
                                 AF.Relu, bias=bc3_sb[:])

            # ---- u / v ----
            psu = pc_pool.tile([H1, S * M], F32, tag="psc")
            psv = pc_pool.tile([H1, S * M], F32, tag="psc")
            nc.tensor.matmul(psu[:], r32(w1a_sb[:]), r32(featc[:]), start=True, stop=True)
            nc.tensor.matmul(psv[:], r32(w1b_sb[:]), r32(featc[:]), start=True, stop=True)
            u_f32 = wpool.tile([H1, S * M], F32)
            v_bf = wpool.tile([H1, S * M], BF16)
            v_f32 = wpool.tile([H1, S * M], F32)
            nc.scalar.activation(u_f32[:], psu[:], AF.Copy)
            nc.vector.tensor_scalar(v_bf[:], psv[:], bg1_sb[:], None, op0=OP.add)
            nc.vector.tensor_scalar(v_f32[:], psv[:], bg1_sb[:], None, op0=OP.add)

            # ---- cls head (overlaps relation; DMA result out early) ----
            fme = wpool.tile([65, S], F32)
            nc.gpsimd.memset(fme[:], 1.0)
            nc.vector.tensor_reduce(
                fme[0:64, :], featc[0:64, :].rearrange("p (i m) -> p i m", m=M),
                axis=mybir.AxisListType.X, op=OP.add)
            psl = pc_pool.tile([S, NCls], F32, tag="psc")
            nc.tensor.matmul(psl[:], r32(fme[:]), r32(wle_sb[:]), start=True, stop=True)
            mx = wpool.tile([S, 1], F32)
            nc.vector.tensor_reduce(mx[:], psl[:], axis=mybir.AxisListType.X, op=OP.max)
            shifted = wpool.tile([S, NCls], F32)
            nc.vector.tensor_scalar(shifted[:], psl[:], mx[:], None, op0=OP.subtract)
            escr = wpool.tile([S, NCls], F32)
            se = wpool.tile([S, 1], F32)
            nc.scalar.activation(escr[:], shifted[:], AF.Exp, accum_out=se[:])
            lse = wpool.tile([S, 1], F32)
            nc.scalar.activation(lse[:], se[:], AF.Ln)
            selscr = wpool.tile([S, NCls], F32)
            sel = wpool.tile([S, 1], F32)
            nc.vector.tensor_tensor(selscr[:], shifted[:], onehot_sb[:], op=OP.mult)
            nc.vector.tensor_reduce(sel[:], selscr[:], axis=mybir.AxisListType.X, op=OP.add)
            clsv = wpool.tile([S, 1], F32)
            nc.vector.tensor_tensor(clsv[:], lse[:], sel[:], op=OP.subtract)
            nc.sync.dma_start(out=out_cls[:], in_=clsv[:])

            # ---- relation stage ----
            # xf_cols[:, (qh*9 + jl*3 + duo) + 18*gh] accumulates one evac op's
            # sum over (16q x 64p); combine gh then qh afterwards.
            xf_cols = wpool.tile([2 * CO, 36], F32)

            # engine assignment pattern for the 32 hdd-gen q ops of each unit
            gps_slots = set(int(i * 32 / KGPS) for i in range(KGPS)) if KGPS else set()
            rest = [s for s in range(32) if s not in gps_slots]
            act_slots = set(rest[int((i + 0.5) * len(rest) / KACT)]
                            for i in range(KACT)) if KACT else set()
            dve_evac_slots = (set(int((i + 0.5) * 6 / KDVE_EVAC)
                                  for i in range(KDVE_EVAC))
                              if KDVE_EVAC else set())

            for jl in range(3):
                for qh in range(2):
                    hdd = hpool.tile([H1, 32, S * M], BF16, tag="hdd")
                    for ql in range(32):
                        q = qh * 32 + ql
                        ucol = u_f32[:, jl * M + q: jl * M + q + 1]
                        if ql in act_slots:
                            nc.scalar.activation(hdd[:, ql, :], v_f32[:],
                                                 AF.Relu, bias=ucol)
                        elif ql in gps_slots:
                            nc.gpsimd.tensor_scalar(hdd[:, ql, :], v_f32[:],
                                                    ucol, 0.0,
                                                    op0=OP.add, op1=OP.max)
                        else:
                            nc.vector.tensor_scalar(hdd[:, ql, :], v_bf[:],
                                                    ucol, 0.0,
                                                    op0=OP.add, op1=OP.max)
                    for duo in range(3):
                        iA, iB = 2 * duo, 2 * duo + 1
                        for gh in range(2):
                            ps = pb_pool.tile([2 * CO, 1024], F32, tag="gps")
                            for q2 in range(2):
                                qg = gh * 2 + q2
                                nc.tensor.matmul(
                                    ps[0:CO, q2 * 512:(q2 + 1) * 512],
                                    wg2_sb[:],
                                    hdd[:, qg * 8:(qg + 1) * 8, iA * M:(iA + 1) * M],
                                    start=True, stop=True)
                                nc.tensor.matmul(
                                    ps[CO:2 * CO, q2 * 512:(q2 + 1) * 512],
                                    wg2_sb[:],
                                    hdd[:, qg * 8:(qg + 1) * 8, iB * M:(iB + 1) * M],
                                    start=True, stop=True,
                                    tile_position=(0, 64))
                            ucol_i = (qh * 9 + jl * 3 + duo) + 18 * gh
                            gscr = spool.tile([2 * CO, 1024], BF16, tag="gscr")
                            if (duo * 2 + gh) in dve_evac_slots:
                                nc.vector._custom_dve(
                                    RELU_BIAS_SUM, out=gscr[:], in0=ps[:],
                                    s0=bg2_sb[:],
                                    accum_out=xf_cols[:, ucol_i:ucol_i + 1])
                            else:
                                nc.scalar.activation(
                                    gscr[:], ps[:], AF.Relu,
                                    bias=bg2_sb[:],
                                    accum_out=xf_cols[:, ucol_i:ucol_i + 1])

            # ---- score head ----
            # sum the two gh-halves, then the two qh-halves
            xf18 = wpool.tile([2 * CO, 18], F32)
            nc.vector.tensor_tensor(
                xf18[:], xf_cols[:, 0:18], xf_cols[:, 18:36], op=OP.add)
            xf_pair = wpool.tile([2 * CO, 9], F32)
            nc.vector.tensor_tensor(
                xf_pair[:], xf18[:, 0:9], xf18[:, 9:18], op=OP.add)
            # assemble xf for both partition halves into one base-0 tile:
            # even i (rows 0:64) -> cols 0:9, odd i -> cols 9:18 (SBUF DMA)
            xf_ext = wpool.tile([CO, 18], F32)
            nc.vector.tensor_copy(xf_ext[:, 0:9], xf_pair[0:CO, :])
            nc.sync.dma_start(out=xf_ext[:, 9:18], in_=xf_pair[CO:2 * CO, :])
            psh1 = pc_pool.tile([16, 18], F32, tag="psc")
            nc.tensor.matmul(psh1[:], r32(wf1d_sb[0:CO, :]),
                             r32(xf_ext[:]), start=True, stop=True)
            h1e = wpool.tile([17, 18], F32)
            nc.gpsimd.memset(h1e[:], 1.0)
            nc.scalar.activation(h1e[0:16, :], psh1[:], AF.Relu, bias=bf1_sb[:])
            psh2 = pc_pool.tile([18, 1], F32, tag="psc")
            nc.tensor.matmul(psh2[:], r32(h1e[:]), r32(wf2e_sb[:]),
                             start=True, stop=True)
            en = wpool.tile([18, 1], F32)
            nc.scalar.activation(en[:], psh2[:], AF.Exp, scale=-1.0)
            ep1 = wpool.tile([18, 1], F32)
            nc.vector.tensor_scalar(ep1[:], en[:], 1.0, None, op0=OP.add)
            sc = wpool.tile([18, 1], F32)
            nc.vector.reciprocal(sc[:], ep1[:])
            nc.sync.dma_start(out=out_scores[:], in_=sc[:])
    nc.compile()
    return nc


_NC_CACHE = None


def _get_nc():
    global _NC_CACHE
    if _NC_CACHE is None:
        _NC_CACHE = _build_nc()
    return _NC_CACHE


def _host_prep(inputs):
    ins = {k: np.asarray(v) for k, v in inputs.items()}
    x = np.concatenate([ins['support_x'], ins['query_x']], axis=1)
    lab = np.concatenate([ins['support_y'], ins['query_y']], axis=1)

    xpad = np.pad(x.astype(np.float32), ((0, 0), (0, 0), (0, 0), (0, 1), (0, 1)))
    win = np.lib.stride_tricks.sliding_window_view(xpad, (3, 3), axis=(3, 4))
    win = win[:, :, :, ::2, ::2]
    patches = win.transpose(0, 2, 5, 6, 1, 3, 4).reshape(B, 27, S, 1024)
    patches = np.ascontiguousarray(patches, np.float32)

    f32 = np.float32
    bf16 = ml_dtypes.bfloat16
    w1 = np.ascontiguousarray(ins['k1'].reshape(32, 27).T, f32).astype(bf16)
    w2 = np.ascontiguousarray(ins['k2'].transpose(1, 2, 3, 0).reshape(32, 9 * 48), f32).astype(bf16)
    w3 = np.ascontiguousarray(ins['k3'].transpose(1, 2, 3, 0).reshape(48, 9 * 64), f32).astype(bf16)

    ii = np.arange(D, dtype=f32) / D
    coord = np.stack([np.broadcast_to(ii[:, None], (D, D)),
                      np.broadcast_to(ii[None, :], (D, D))]).reshape(2, M)
    coords = np.ascontiguousarray(np.tile(coord, (1, S)), f32).astype(bf16)

    onehots = np.zeros((B, S, NCls), f32)
    for b in range(B):
        onehots[b, np.arange(S), lab[b]] = 1.0

    Wg1 = ins['Wg1'].astype(f32)
    common = dict(
        w1=w1, w2=w2, w3=w3,
        bc1=np.ascontiguousarray(ins['bc1'].reshape(32, 1), f32),
        bc2=np.ascontiguousarray(ins['bc2'].reshape(48, 1), f32),
        bc3=np.ascontiguousarray(ins['bc3'].reshape(64, 1), f32),
        coords=coords,
        wle=np.ascontiguousarray(
            np.vstack([ins['Wlog'].astype(f32) / M, ins['blog'][None, :].astype(f32)])),
        w1a=np.ascontiguousarray(Wg1[:C2]).astype(bf16),
        w1b=np.ascontiguousarray(Wg1[C2:]).astype(bf16),
        bg1=np.ascontiguousarray(ins['bg1'].reshape(H1, 1), f32),
        wg2=np.ascontiguousarray(ins['Wg2'], f32).astype(ml_dtypes.bfloat16),
        bg2_2=np.ascontiguousarray(np.tile(ins['bg2'].astype(f32), 2).reshape(2 * CO, 1)),
        wf1d=np.ascontiguousarray(
            np.vstack([ins['Wf1'].astype(f32), ins['Wf1'].astype(f32)])),
        bf1=np.ascontiguousarray(ins['bf1'].reshape(16, 1), f32),
        wf2e=np.ascontiguousarray(
            np.vstack([ins['Wf2'].astype(f32), ins['bf2'].reshape(1, 1).astype(f32)])),
    )
    in_maps = []
    for core in range(N_CORES):
        b, half = core // 2, core % 2
        # odd cores see images in rotated order so the program's local
        # j in {0,1,2} maps to global j in {3,4,5}
        perm = (0, 1, 2, 3, 4, 5) if half == 0 else (3, 4, 5, 0, 1, 2)
        m = dict(common)
        m['patches'] = np.ascontiguousarray(patches[b][:, perm, :]).astype(ml_dtypes.bfloat16)
        m['onehot'] = np.ascontiguousarray(onehots[b][list(perm)])
        in_maps.append(m)
    return in_maps, lab


def _host_post(results, lab):
    P = np.zeros((B, S, S), np.float32)
    cls_terms = np.zeros((B, S), np.float32)
    for core in range(N_CORES):
        b, half = core // 2, core % 2
        perm = (0, 1, 2, 3, 4, 5) if half == 0 else (3, 4, 5, 0, 1, 2)
        sc = results[core]["scores"].reshape(18)
        # score col layout: k < 9: (jl = k//3, duo = k%3, i = 2*duo);
        #                   k >= 9: i = 2*duo + 1
        for k in range(18):
            kk = k % 9
            jl, duo = kk // 3, kk % 3
            i = 2 * duo + (1 if k >= 9 else 0)
            P[b, perm[i], perm[jl]] = sc[k]
        if half == 0:
            cls_terms[b] = results[core]["clsv"].reshape(S)
    cls_loss = np.float32(cls_terms.mean())
    y = (lab[:, :, None] == lab[:, None, :]).astype(np.float32)
    Pt = P.transpose(0, 2, 1)
    sym, anti = np.float32(0.5) * (P + Pt), np.float32(0.5) * (P - Pt)
    sym_n = np.sqrt((sym ** 2).sum(axis=(1, 2)))
    anti_n = np.sqrt((anti ** 2).sum(axis=(1, 2)))
    sym_loss = np.float32(((sym_n - anti_n) / (sym_n + anti_n)).mean())
    euc_loss = np.float32(((P - y) ** 2).mean())
    rn_loss = np.float32(euc_loss - np.float32(0.1) * sym_loss)
    return np.float32(cls_loss), np.float32(rn_loss), np.float32(sym_loss)


def run_spmd(inputs, trace=False, **kwargs):
    nc = _get_nc()
    in_maps, lab = _host_prep(inputs)
    res = run_bass_kernel_spmd(nc, in_maps, list(range(N_CORES)),
                               trace=trace, **kwargs)
    return _host_post(res.results, lab), res


def kernel(**inputs):
    out, _ = run_spmd(inputs)
    return out


# revision 25
# speedup vs baseline: 4.1869x; 1.1272x over previous
"""Trainium2 Bass kernel for nn_Meta_67078799229377 (relation-network meta-learner).

Sharding: 8 cores = 4 batch elements x 2 halves of the relation-j axis.
Each core runs the full backbone for its batch element's 6 images, then the
relation network for its 18 (i, j) pairs, fully fused on-chip (the
[s,s,m,m,128] tensor never exists in HBM). Host code only reshapes/shards
inputs and combines 144 scores + 24 per-sample CE terms into the 3 scalar
losses.

v2 changes vs v1:
 - input DMAs reordered (patches first) and spread across engine queues
 - consolidated pad memsets, early ACT table prefetch (exp/ln)
 - hdd generation split across DVE/ACT/GPSIMD (env-tunable)
 - g evacuation in FD=2048 ops split ACT/DVE, PSUM tiles [128,2048]
 - score head uses two matmuls on xf partition halves (no SBUF-shift DMA)
 - cls output DMA issued as soon as ready
"""
import os
import numpy as np
import ml_dtypes

import concourse.bass as bass
import concourse.mybir as mybir
import concourse.tile as tile
from concourse import bacc
from concourse.bass_utils import run_bass_kernel_spmd

F32 = mybir.dt.float32
BF16 = mybir.dt.bfloat16
AF = mybir.ActivationFunctionType
OP = mybir.AluOpType


def _register_relu_bias_sum():
    """Custom DVE op: out = relu(in0 + s0), accum_out = sum(out).

    Fuses the g-evacuation (relu + bias + spatial-sum) into one Vector-engine
    instruction; stock tensor_scalar repurposes op1 as the reduce op when
    accum_out is attached, so it cannot express this.
    """
    from concourse import dve_ops
    from concourse.dve_spec import Spec, Src0, C0, Zero, relu, lower
    from concourse.dve_spec import _has_src1 as has_src1
    from concourse.dve_uop import DveOpSpec
    from operator import add as op_add

    name = "RELU_BIAS_SUM_ANT"
    for op in dve_ops.OPS:
        if op.name == name:
            return op

    def _ref(in0, in1, c0, c1, c2):
        b = np.maximum(in0.astype(np.float32) + c0, 0).astype(np.float32)
        return b, b.reshape(b.shape[0], -1).sum(axis=-1, keepdims=True)

    row = max(dve_ops._SUB_OPCODE_FOR_NAME.values()) + 1
    assert row < 0x20
    dve_ops._SUB_OPCODE_FOR_NAME[name] = row
    spec = Spec(body=relu(Src0 + C0), accum=op_add, accum_init=Zero,
                reference=_ref)
    shas = {}
    for ver in ("v3", "v4"):
        dspec = DveOpSpec(name=name, opcode=row, uops=lower(spec, ver=ver),
                          rd1_en=has_src1(spec))
        shas[ver] = dspec.sha(ver)
    op = dve_ops.DveOp(name, spec, subdim=False, uops_sha=shas)
    dve_ops.OPS.append(op)
    dve_ops.CUSTOM_DVE_SPECS[name] = spec
    return op


RELU_BIAS_SUM = None
if int(os.environ.get("KDVE_EVAC", "0")) > 0:
    RELU_BIAS_SUM = _register_relu_bias_sum()

B, S, D = 4, 6, 8
M = D * D            # 64 spatial positions
C2 = 66              # 64 channels + 2 coord channels
H1 = 128             # g-MLP hidden
CO = 64              # g-MLP out
NCls = 64
N_CORES = 8

# hdd-gen engine split per unit (32 q-ops): gpsimd + act counts; rest on DVE.
KGPS = int(os.environ.get("KGPS", "0"))
KACT = int(os.environ.get("KACT", "1"))
# of the 6 per-unit evacuation ops, how many go to DVE (rest on ACT)
KDVE_EVAC = int(os.environ.get("KDVE_EVAC", "0"))
# which engines take non-sync const DMAs: "sync" | "spread"
KDMA = os.environ.get("KDMA", "sync")


def _build_nc():
    nc = bacc.Bacc("TRN2", target_bir_lowering=False, debug=False,
                   num_devices=N_CORES)

    din = {}
    def dram_in(name, shape, dtype=F32):
        din[name] = nc.dram_tensor(name, list(shape), dtype, kind="ExternalInput")
        return din[name]

    x_patches = dram_in("patches", [27, S, 1024], BF16)
    x_w1 = dram_in("w1", [27, 32], BF16)
    x_w2 = dram_in("w2", [32, 9 * 48], BF16)
    x_w3 = dram_in("w3", [48, 9 * 64], BF16)
    x_bc1 = dram_in("bc1", [32, 1])
    x_bc2 = dram_in("bc2", [48, 1])
    x_bc3 = dram_in("bc3", [64, 1])
    x_coords = dram_in("coords", [2, S * M], BF16)
    x_wle = dram_in("wle", [65, NCls])
    x_onehot = dram_in("onehot", [S, NCls])
    x_w1a = dram_in("w1a", [C2, H1], BF16)
    x_w1b = dram_in("w1b", [C2, H1], BF16)
    x_bg1 = dram_in("bg1", [H1, 1])
    x_wg2 = dram_in("wg2", [H1, CO], BF16)
    x_bg2 = dram_in("bg2_2", [2 * CO, 1])
    x_wf1d = dram_in("wf1d", [2 * CO, 16])   # Wf1 stacked twice (for row halves)
    x_bf1 = dram_in("bf1", [16, 1])
    x_wf2e = dram_in("wf2e", [17, 1])

    out_scores = nc.dram_tensor("scores", [18, 1], F32, kind="ExternalOutput")
    # clsv[:, 0] = sum(exp(shifted)); clsv[:, 1] = selected shifted logit.
    # Host computes ln(se) - sel, so the device never needs the Ln table.
    out_cls = nc.dram_tensor("clsv", [S, 2], F32, kind="ExternalOutput")

    with tile.TileContext(nc) as tc:
        with (
            tc.tile_pool(name="const", bufs=1) as cpool,
            tc.tile_pool(name="work", bufs=1) as wpool,
            tc.tile_pool(name="patch", bufs=1) as ppool,
            tc.tile_pool(name="hdd", bufs=4) as hpool,
            tc.tile_pool(name="gscr", bufs=3) as spool,
            tc.tile_pool(name="psmall", bufs=2, space="PSUM") as pc_pool,
            tc.tile_pool(name="pbig", bufs=2, space="PSUM") as pb_pool,
        ):
            # ---- inputs to SBUF; order matters: conv-critical first on sync ----
            def c_tile(src, shape, dtype=F32, eng=None):
                t = cpool.tile(list(shape), dtype, tag=src.name)
                (eng or nc.sync).dma_start(out=t[:], in_=src[:])
                return t

            patches_sb = ppool.tile([27, S, 1024], BF16)
            nc.sync.dma_start(out=patches_sb[:], in_=x_patches[:])
            w1_sb = c_tile(x_w1, [27, 32], BF16)
            bc1_sb = c_tile(x_bc1, [32, 1])
            w2_sb = c_tile(x_w2, [32, 9 * 48], BF16)
            bc2_sb = c_tile(x_bc2, [48, 1])
            w3_sb = c_tile(x_w3, [48, 9 * 64], BF16)
            bc3_sb = c_tile(x_bc3, [64, 1])

            featc = wpool.tile([C2, S * M], BF16)
            # non-critical consts; optionally spread across the scalar queue
            # (only SP/Activation/gpsimd can initiate DMAs)
            alt = nc.scalar if KDMA == "spread" else nc.sync
            nc.sync.dma_start(out=featc[64:66, :], in_=x_coords[:])
            w1a_sb = c_tile(x_w1a, [C2, H1], BF16, eng=alt)
            w1b_sb = c_tile(x_w1b, [C2, H1], BF16, eng=alt)
            bg1_sb = c_tile(x_bg1, [H1, 1], eng=alt)
            wg2_sb = c_tile(x_wg2, [H1, CO], BF16, eng=alt)
            bg2_sb = c_tile(x_bg2, [2 * CO, 1], eng=alt)
            wle_sb = c_tile(x_wle, [65, NCls])
            onehot_sb = c_tile(x_onehot, [S, NCls])
            wf1d_sb = c_tile(x_wf1d, [2 * CO, 16])
            bf1_sb = c_tile(x_bf1, [16, 1])
            wf2e_sb = c_tile(x_wf2e, [17, 1])

            def r32(ap):
                return ap

            # ---- conv1: [27]->[32], 64x64 -> 32x32 (stride 2, im2col'd) ----
            c1sb = wpool.tile([32, S, 33, 33], BF16)
            c2sb = wpool.tile([48, S, 17, 17], BF16)
            for img in range(S):
                nc.gpsimd.memset(c1sb[:, img, 32, :], 0.0)
                nc.gpsimd.memset(c1sb[:, img, 0:32, 32], 0.0)
                nc.gpsimd.memset(c2sb[:, img, 16, :], 0.0)
                nc.gpsimd.memset(c2sb[:, img, 0:16, 16], 0.0)

            for img in range(S):
                for h in range(2):
                    ps1 = pc_pool.tile([32, 16, 32], F32, tag="psc")
                    nc.tensor.matmul(
                        ps1[:].rearrange("p a b -> p (a b)"),
                        r32(w1_sb[:]),
                        r32(patches_sb[:, img, h * 512:(h + 1) * 512]),
                        start=True, stop=True)
                    # relu(x + bc1) -> padded layout; DVE (ACT is busier later)
                    out_ap = c1sb[:, img, h * 16:(h + 1) * 16, 0:32]
                    if img % 3 == 2:
                        nc.scalar.activation(out_ap, ps1[:], AF.Relu, bias=bc1_sb[:])
                    else:
                        nc.vector.tensor_scalar(out_ap, ps1[:], bc1_sb[:], 0.0,
                                                op0=OP.add, op1=OP.max)

            # ---- conv2: [32]->[48], 32x32 -> 16x16 ----
            for ip in range(3):      # image pairs
                ps2 = pc_pool.tile([48, 2, 16, 16], F32, tag="psc")
                for k, (dy, dx) in enumerate((dy, dx) for dy in range(3) for dx in range(3)):
                    nc.tensor.matmul(
                        ps2[:],
                        r32(w2_sb[:, k * 48:(k + 1) * 48]),
                        r32(c1sb[:, 2 * ip:2 * ip + 2, dy:dy + 31:2, dx:dx + 31:2]),
                        start=(k == 0), stop=(k == 8))
                out_ap = c2sb[:, 2 * ip:2 * ip + 2, 0:16, 0:16]
                if ip % 2 == 0:
                    nc.scalar.activation(out_ap, ps2[:], AF.Relu, bias=bc2_sb[:])
                else:
                    nc.vector.tensor_scalar(out_ap, ps2[:], bc2_sb[:], 0.0,
                                            op0=OP.add, op1=OP.max)

            # ---- conv3: [48]->[64], 16x16 -> 8x8 ----
            ps3 = pc_pool.tile([64, S, D, D], F32, tag="psc")
            for k, (dy, dx) in enumerate((dy, dx) for dy in range(3) for dx in range(3)):
                nc.tensor.matmul(
                    ps3[:],
                    r32(w3_sb[:, k * 64:(k + 1) * 64]),
                    r32(c2sb[:, :, dy:dy + 15:2, dx:dx + 15:2]),
                    start=(k == 0), stop=(k == 8))
            nc.scalar.activation(featc[0:64, :].rearrange("p (i m) -> p i m", m=M),
                                 ps3[:].rearrange("p i a b -> p i (a b)"),
                                 AF.Relu, bias=bc3_sb[:])

            # ---- u / v ----
            psu = pc_pool.tile([H1, S * M], F32, tag="psc")
            psv = pc_pool.tile([H1, S * M], F32, tag="psc")
            nc.tensor.matmul(psu[:], r32(w1a_sb[:]), r32(featc[:]), start=True, stop=True)
            nc.tensor.matmul(psv[:], r32(w1b_sb[:]), r32(featc[:]), start=True, stop=True)
            u_f32 = wpool.tile([H1, S * M], F32)
            v_bf = wpool.tile([H1, S * M], BF16)
            v_f32 = wpool.tile([H1, S * M], F32)
            nc.scalar.activation(u_f32[:], psu[:], AF.Copy)
            nc.vector.tensor_scalar(v_bf[:], psv[:], bg1_sb[:], None, op0=OP.add)
            nc.vector.tensor_scalar(v_f32[:], psv[:], bg1_sb[:], None, op0=OP.add)

            # ---- cls head (overlaps relation; DMA result out early) ----
            fme = wpool.tile([65, S], F32)
            nc.gpsimd.memset(fme[:], 1.0)
            nc.vector.tensor_reduce(
                fme[0:64, :], featc[0:64, :].rearrange("p (i m) -> p i m", m=M),
                axis=mybir.AxisListType.X, op=OP.add)
            psl = pc_pool.tile([S, NCls], F32, tag="psc")
            nc.tensor.matmul(psl[:], r32(fme[:]), r32(wle_sb[:]), start=True, stop=True)
            mx = wpool.tile([S, 1], F32)
            nc.vector.tensor_reduce(mx[:], psl[:], axis=mybir.AxisListType.X, op=OP.max)
            shifted = wpool.tile([S, NCls], F32)
            nc.vector.tensor_scalar(shifted[:], psl[:], mx[:], None, op0=OP.subtract)
            escr = wpool.tile([S, NCls], F32)
            se = wpool.tile([S, 1], F32)
            nc.scalar.activation(escr[:], shifted[:], AF.Exp, accum_out=se[:])
            selscr = wpool.tile([S, NCls], F32)
            sel = wpool.tile([S, 1], F32)
            nc.vector.tensor_tensor(selscr[:], shifted[:], onehot_sb[:], op=OP.mult)
            nc.vector.tensor_reduce(sel[:], selscr[:], axis=mybir.AxisListType.X, op=OP.add)
            nc.sync.dma_start(out=out_cls[:, 0:1], in_=se[:])
            nc.sync.dma_start(out=out_cls[:, 1:2], in_=sel[:])

            # ---- relation stage ----
            # xf_cols[:, (qh*9 + jl*3 + duo) + 18*gh] accumulates one evac op's
            # sum over (16q x 64p); combine gh then qh afterwards.
            xf_cols = wpool.tile([2 * CO, 36], F32)

            # engine assignment pattern for the 32 hdd-gen q ops of each unit
            gps_slots = set(int(i * 32 / KGPS) for i in range(KGPS)) if KGPS else set()
            rest = [s for s in range(32) if s not in gps_slots]
            act_slots = set(rest[int((i + 0.5) * len(rest) / KACT)]
                            for i in range(KACT)) if KACT else set()
            dve_evac_slots = (set(int((i + 0.5) * 6 / KDVE_EVAC)
                                  for i in range(KDVE_EVAC))
                              if KDVE_EVAC else set())

            for jl in range(3):
                for qh in range(2):
                    hdd = hpool.tile([H1, 32, S * M], BF16, tag="hdd")
                    for ql in range(32):
                        q = qh * 32 + ql
                        ucol = u_f32[:, jl * M + q: jl * M + q + 1]
                        if ql in act_slots:
                            nc.scalar.activation(hdd[:, ql, :], v_f32[:],
                                                 AF.Relu, bias=ucol)
                        elif ql in gps_slots:
                            nc.gpsimd.tensor_scalar(hdd[:, ql, :], v_f32[:],
                                                    ucol, 0.0,
                                                    op0=OP.add, op1=OP.max)
                        else:
                            nc.vector.tensor_scalar(hdd[:, ql, :], v_bf[:],
                                                    ucol, 0.0,
                                                    op0=OP.add, op1=OP.max)
                    for duo in range(3):
                        iA, iB = 2 * duo, 2 * duo + 1
                        for gh in range(2):
                            ps = pb_pool.tile([2 * CO, 1024], F32, tag="gps")
                            for q2 in range(2):
                                qg = gh * 2 + q2
                                nc.tensor.matmul(
                                    ps[0:CO, q2 * 512:(q2 + 1) * 512],
                                    wg2_sb[:],
                                    hdd[:, qg * 8:(qg + 1) * 8, iA * M:(iA + 1) * M],
                                    start=True, stop=True)
                                nc.tensor.matmul(
                                    ps[CO:2 * CO, q2 * 512:(q2 + 1) * 512],
                                    wg2_sb[:],
                                    hdd[:, qg * 8:(qg + 1) * 8, iB * M:(iB + 1) * M],
                                    start=True, stop=True,
                                    tile_position=(0, 64))
                            ucol_i = (qh * 9 + jl * 3 + duo) + 18 * gh
                            gscr = spool.tile([2 * CO, 1024], BF16, tag="gscr")
                            if (duo * 2 + gh) in dve_evac_slots:
                                nc.vector._custom_dve(
                                    RELU_BIAS_SUM, out=gscr[:], in0=ps[:],
                                    s0=bg2_sb[:],
                                    accum_out=xf_cols[:, ucol_i:ucol_i + 1])
                            else:
                                nc.scalar.activation(
                                    gscr[:], ps[:], AF.Relu,
                                    bias=bg2_sb[:],
                                    accum_out=xf_cols[:, ucol_i:ucol_i + 1])

            # ---- score head ----
            # sum the two gh-halves, then the two qh-halves
            xf18 = wpool.tile([2 * CO, 18], F32)
            nc.vector.tensor_tensor(
                xf18[:], xf_cols[:, 0:18], xf_cols[:, 18:36], op=OP.add)
            xf_pair = wpool.tile([2 * CO, 9], F32)
            nc.vector.tensor_tensor(
                xf_pair[:], xf18[:, 0:9], xf18[:, 9:18], op=OP.add)
            # assemble xf for both partition halves into one base-0 tile:
            # even i (rows 0:64) -> cols 0:9, odd i -> cols 9:18 (SBUF DMA)
            xf_ext = wpool.tile([CO, 18], F32)
            nc.vector.tensor_copy(xf_ext[:, 0:9], xf_pair[0:CO, :])
            nc.sync.dma_start(out=xf_ext[:, 9:18], in_=xf_pair[CO:2 * CO, :])
            psh1 = pc_pool.tile([16, 18], F32, tag="psc")
            nc.tensor.matmul(psh1[:], r32(wf1d_sb[0:CO, :]),
                             r32(xf_ext[:]), start=True, stop=True)
            h1e = wpool.tile([17, 18], F32)
            nc.gpsimd.memset(h1e[:], 1.0)
            nc.scalar.activation(h1e[0:16, :], psh1[:], AF.Relu, bias=bf1_sb[:])
            psh2 = pc_pool.tile([18, 1], F32, tag="psc")
            nc.tensor.matmul(psh2[:], r32(h1e[:]), r32(wf2e_sb[:]),
                             start=True, stop=True)
            en = wpool.tile([18, 1], F32)
            nc.scalar.activation(en[:], psh2[:], AF.Exp, scale=-1.0)
            ep1 = wpool.tile([18, 1], F32)
            nc.vector.tensor_scalar(ep1[:], en[:], 1.0, None, op0=OP.add)
            sc = wpool.tile([18, 1], F32)
            nc.vector.reciprocal(sc[:], ep1[:])
            nc.sync.dma_start(out=out_scores[:], in_=sc[:])
    nc.compile()
    return nc


_NC_CACHE = None


def _get_nc():
    global _NC_CACHE
    if _NC_CACHE is None:
        _NC_CACHE = _build_nc()
    return _NC_CACHE


def _host_prep(inputs):
    ins = {k: np.asarray(v) for k, v in inputs.items()}
    x = np.concatenate([ins['support_x'], ins['query_x']], axis=1)
    lab = np.concatenate([ins['support_y'], ins['query_y']], axis=1)

    xpad = np.pad(x.astype(np.float32), ((0, 0), (0, 0), (0, 0), (0, 1), (0, 1)))
    win = np.lib.stride_tricks.sliding_window_view(xpad, (3, 3), axis=(3, 4))
    win = win[:, :, :, ::2, ::2]
    patches = win.transpose(0, 2, 5, 6, 1, 3, 4).reshape(B, 27, S, 1024)
    patches = np.ascontiguousarray(patches, np.float32)

    f32 = np.float32
    bf16 = ml_dtypes.bfloat16
    w1 = np.ascontiguousarray(ins['k1'].reshape(32, 27).T, f32).astype(bf16)
    w2 = np.ascontiguousarray(ins['k2'].transpose(1, 2, 3, 0).reshape(32, 9 * 48), f32).astype(bf16)
    w3 = np.ascontiguousarray(ins['k3'].transpose(1, 2, 3, 0).reshape(48, 9 * 64), f32).astype(bf16)

    ii = np.arange(D, dtype=f32) / D
    coord = np.stack([np.broadcast_to(ii[:, None], (D, D)),
                      np.broadcast_to(ii[None, :], (D, D))]).reshape(2, M)
    coords = np.ascontiguousarray(np.tile(coord, (1, S)), f32).astype(bf16)

    onehots = np.zeros((B, S, NCls), f32)
    for b in range(B):
        onehots[b, np.arange(S), lab[b]] = 1.0

    Wg1 = ins['Wg1'].astype(f32)
    common = dict(
        w1=w1, w2=w2, w3=w3,
        bc1=np.ascontiguousarray(ins['bc1'].reshape(32, 1), f32),
        bc2=np.ascontiguousarray(ins['bc2'].reshape(48, 1), f32),
        bc3=np.ascontiguousarray(ins['bc3'].reshape(64, 1), f32),
        coords=coords,
        wle=np.ascontiguousarray(
            np.vstack([ins['Wlog'].astype(f32) / M, ins['blog'][None, :].astype(f32)])),
        w1a=np.ascontiguousarray(Wg1[:C2]).astype(bf16),
        w1b=np.ascontiguousarray(Wg1[C2:]).astype(bf16),
        bg1=np.ascontiguousarray(ins['bg1'].reshape(H1, 1), f32),
        wg2=np.ascontiguousarray(ins['Wg2'], f32).astype(ml_dtypes.bfloat16),
        bg2_2=np.ascontiguousarray(np.tile(ins['bg2'].astype(f32), 2).reshape(2 * CO, 1)),
        wf1d=np.ascontiguousarray(
            np.vstack([ins['Wf1'].astype(f32), ins['Wf1'].astype(f32)])),
        bf1=np.ascontiguousarray(ins['bf1'].reshape(16, 1), f32),
        wf2e=np.ascontiguousarray(
            np.vstack([ins['Wf2'].astype(f32), ins['bf2'].reshape(1, 1).astype(f32)])),
    )
    in_maps = []
    for core in range(N_CORES):
        b, half = core // 2, core % 2
        # odd cores see images in rotated order so the program's local
        # j in {0,1,2} maps to global j in {3,4,5}
        perm = (0, 1, 2, 3, 4, 5) if half == 0 else (3, 4, 5, 0, 1, 2)
        m = dict(common)
        m['patches'] = np.ascontiguousarray(patches[b][:, perm, :]).astype(ml_dtypes.bfloat16)
        m['onehot'] = np.ascontiguousarray(onehots[b][list(perm)])
        in_maps.append(m)
    return in_maps, lab


def _host_post(results, lab):
    P = np.zeros((B, S, S), np.float32)
    cls_terms = np.zeros((B, S), np.float32)
    for core in range(N_CORES):
        b, half = core // 2, core % 2
        perm = (0, 1, 2, 3, 4, 5) if half == 0 else (3, 4, 5, 0, 1, 2)
        sc = results[core]["scores"].reshape(18)
        # score col layout: k < 9: (jl = k//3, duo = k%3, i = 2*duo);
        #                   k >= 9: i = 2*duo + 1
        for k in range(18):
            kk = k % 9
            jl, duo = kk // 3, kk % 3
            i = 2 * duo + (1 if k >= 9 else 0)
            P[b, perm[i], perm[jl]] = sc[k]
        if half == 0:
            sesel = results[core]["clsv"].reshape(S, 2)
            cls_terms[b] = np.log(sesel[:, 0]) - sesel[:, 1]
    cls_loss = np.float32(cls_terms.mean())
    y = (lab[:, :, None] == lab[:, None, :]).astype(np.float32)
    Pt = P.transpose(0, 2, 1)
    sym, anti = np.float32(0.5) * (P + Pt), np.float32(0.5) * (P - Pt)
    sym_n = np.sqrt((sym ** 2).sum(axis=(1, 2)))
    anti_n = np.sqrt((anti ** 2).sum(axis=(1, 2)))
    sym_loss = np.float32(((sym_n - anti_n) / (sym_n + anti_n)).mean())
    euc_loss = np.float32(((P - y) ** 2).mean())
    rn_loss = np.float32(euc_loss - np.float32(0.1) * sym_loss)
    return np.float32(cls_loss), np.float32(rn_loss), np.float32(sym_loss)


def run_spmd(inputs, trace=False, **kwargs):
    nc = _get_nc()
    in_maps, lab = _host_prep(inputs)
    res = run_bass_kernel_spmd(nc, in_maps, list(range(N_CORES)),
                               trace=trace, **kwargs)
    return _host_post(res.results, lab), res


def kernel(**inputs):
    out, _ = run_spmd(inputs)
    return out


# revision 26
# speedup vs baseline: 4.2666x; 1.0190x over previous
"""Trainium2 Bass kernel for nn_Meta_67078799229377 (relation-network meta-learner).

Sharding: 8 cores = 4 batch elements x 2 halves of the relation-j axis.
Each core runs the full backbone for its batch element's 6 images, then the
relation network for its 18 (i, j) pairs, fully fused on-chip (the
[s,s,m,m,128] tensor never exists in HBM). Host code only reshapes/shards
inputs and combines 144 scores + 24 per-sample CE terms into the 3 scalar
losses.

v2 changes vs v1:
 - input DMAs reordered (patches first) and spread across engine queues
 - consolidated pad memsets, early ACT table prefetch (exp/ln)
 - hdd generation split across DVE/ACT/GPSIMD (env-tunable)
 - g evacuation in FD=2048 ops split ACT/DVE, PSUM tiles [128,2048]
 - score head uses two matmuls on xf partition halves (no SBUF-shift DMA)
 - cls output DMA issued as soon as ready
"""
import os
import numpy as np
import ml_dtypes

import concourse.bass as bass
import concourse.mybir as mybir
import concourse.tile as tile
from concourse import bacc
from concourse.bass_utils import run_bass_kernel_spmd

F32 = mybir.dt.float32
BF16 = mybir.dt.bfloat16
AF = mybir.ActivationFunctionType
OP = mybir.AluOpType


def _register_relu_bias_sum():
    """Custom DVE op: out = relu(in0 + s0), accum_out = sum(out).

    Fuses the g-evacuation (relu + bias + spatial-sum) into one Vector-engine
    instruction; stock tensor_scalar repurposes op1 as the reduce op when
    accum_out is attached, so it cannot express this.
    """
    from concourse import dve_ops
    from concourse.dve_spec import Spec, Src0, C0, Zero, relu, lower
    from concourse.dve_spec import _has_src1 as has_src1
    from concourse.dve_uop import DveOpSpec
    from operator import add as op_add

    name = "RELU_BIAS_SUM_ANT"
    for op in dve_ops.OPS:
        if op.name == name:
            return op

    def _ref(in0, in1, c0, c1, c2):
        b = np.maximum(in0.astype(np.float32) + c0, 0).astype(np.float32)
        return b, b.reshape(b.shape[0], -1).sum(axis=-1, keepdims=True)

    row = max(dve_ops._SUB_OPCODE_FOR_NAME.values()) + 1
    assert row < 0x20
    dve_ops._SUB_OPCODE_FOR_NAME[name] = row
    spec = Spec(body=relu(Src0 + C0), accum=op_add, accum_init=Zero,
                reference=_ref)
    shas = {}
    for ver in ("v3", "v4"):
        dspec = DveOpSpec(name=name, opcode=row, uops=lower(spec, ver=ver),
                          rd1_en=has_src1(spec))
        shas[ver] = dspec.sha(ver)
    op = dve_ops.DveOp(name, spec, subdim=False, uops_sha=shas)
    dve_ops.OPS.append(op)
    dve_ops.CUSTOM_DVE_SPECS[name] = spec
    return op


RELU_BIAS_SUM = None
if int(os.environ.get("KDVE_EVAC", "0")) > 0:
    RELU_BIAS_SUM = _register_relu_bias_sum()

B, S, D = 4, 6, 8
M = D * D            # 64 spatial positions
C2 = 66              # 64 channels + 2 coord channels
H1 = 128             # g-MLP hidden
CO = 64              # g-MLP out
NCls = 64
N_CORES = 8

# hdd-gen engine split per unit (32 q-ops): gpsimd + act counts; rest on DVE.
KGPS = int(os.environ.get("KGPS", "0"))
KACT = int(os.environ.get("KACT", "1"))
# of the 6 per-unit evacuation ops, how many go to DVE (rest on ACT)
KDVE_EVAC = int(os.environ.get("KDVE_EVAC", "0"))
# which engines take non-sync const DMAs: "sync" | "spread"
KDMA = os.environ.get("KDMA", "sync")


def _build_nc():
    nc = bacc.Bacc("TRN2", target_bir_lowering=False, debug=False,
                   num_devices=N_CORES)

    din = {}
    def dram_in(name, shape, dtype=F32):
        din[name] = nc.dram_tensor(name, list(shape), dtype, kind="ExternalInput")
        return din[name]

    x_patches = dram_in("patches", [27, S, 1024], BF16)
    x_w1 = dram_in("w1", [27, 32], BF16)
    x_w2 = dram_in("w2", [32, 9 * 48], BF16)
    x_w3 = dram_in("w3", [48, 9 * 64], BF16)
    x_bc1 = dram_in("bc1", [32, 1])
    x_bc2 = dram_in("bc2", [48, 1])
    x_bc3 = dram_in("bc3", [64, 1])
    x_coords = dram_in("coords", [2, S * M], BF16)
    x_wle = dram_in("wle", [65, NCls])
    x_onehot = dram_in("onehot", [S, NCls])
    x_w1a = dram_in("w1a", [C2, H1], BF16)
    x_w1b = dram_in("w1b", [C2, H1], BF16)
    x_bg1 = dram_in("bg1", [H1, 1])
    x_wg2 = dram_in("wg2", [H1, CO], BF16)
    x_bg2 = dram_in("bg2_2", [2 * CO, 1])
    x_wf1d = dram_in("wf1d", [2 * CO, 16])   # Wf1 stacked twice (for row halves)
    x_bf1 = dram_in("bf1", [16, 1])
    x_wf2e = dram_in("wf2e", [17, 1])

    out_scores = nc.dram_tensor("scores", [18, 1], F32, kind="ExternalOutput")
    # clsv[:, 0] = sum(exp(shifted)); clsv[:, 1] = selected shifted logit.
    # Host computes ln(se) - sel, so the device never needs the Ln table.
    out_cls = nc.dram_tensor("clsv", [S, 2], F32, kind="ExternalOutput")

    with tile.TileContext(nc) as tc:
        with (
            tc.tile_pool(name="const", bufs=1) as cpool,
            tc.tile_pool(name="work", bufs=1) as wpool,
            tc.tile_pool(name="patch", bufs=1) as ppool,
            tc.tile_pool(name="hdd", bufs=4) as hpool,
            tc.tile_pool(name="gscr", bufs=3) as spool,
            tc.tile_pool(name="psmall", bufs=2, space="PSUM") as pc_pool,
            tc.tile_pool(name="pbig", bufs=3, space="PSUM") as pb_pool,
        ):
            # ---- inputs to SBUF; order matters: conv-critical first on sync ----
            def c_tile(src, shape, dtype=F32, eng=None):
                t = cpool.tile(list(shape), dtype, tag=src.name)
                (eng or nc.sync).dma_start(out=t[:], in_=src[:])
                return t

            patches_sb = ppool.tile([27, S, 1024], BF16)
            nc.sync.dma_start(out=patches_sb[:], in_=x_patches[:])
            w1_sb = c_tile(x_w1, [27, 32], BF16)
            bc1_sb = c_tile(x_bc1, [32, 1])
            w2_sb = c_tile(x_w2, [32, 9 * 48], BF16)
            bc2_sb = c_tile(x_bc2, [48, 1])
            w3_sb = c_tile(x_w3, [48, 9 * 64], BF16)
            bc3_sb = c_tile(x_bc3, [64, 1])

            featc = wpool.tile([C2, S * M], BF16)
            # non-critical consts; optionally spread across the scalar queue
            # (only SP/Activation/gpsimd can initiate DMAs)
            alt = nc.scalar if KDMA == "spread" else nc.sync
            nc.sync.dma_start(out=featc[64:66, :], in_=x_coords[:])
            w1a_sb = c_tile(x_w1a, [C2, H1], BF16, eng=alt)
            w1b_sb = c_tile(x_w1b, [C2, H1], BF16, eng=alt)
            bg1_sb = c_tile(x_bg1, [H1, 1], eng=alt)
            wg2_sb = c_tile(x_wg2, [H1, CO], BF16, eng=alt)
            bg2_sb = c_tile(x_bg2, [2 * CO, 1], eng=alt)
            wle_sb = c_tile(x_wle, [65, NCls])
            onehot_sb = c_tile(x_onehot, [S, NCls])
            wf1d_sb = c_tile(x_wf1d, [2 * CO, 16])
            bf1_sb = c_tile(x_bf1, [16, 1])
            wf2e_sb = c_tile(x_wf2e, [17, 1])

            def r32(ap):
                return ap

            # ---- conv1: [27]->[32], 64x64 -> 32x32 (stride 2, im2col'd) ----
            c1sb = wpool.tile([32, S, 33, 33], BF16)
            c2sb = wpool.tile([48, S, 17, 17], BF16)
            for img in range(S):
                nc.gpsimd.memset(c1sb[:, img, 32, :], 0.0)
                nc.gpsimd.memset(c1sb[:, img, 0:32, 32], 0.0)
                nc.gpsimd.memset(c2sb[:, img, 16, :], 0.0)
                nc.gpsimd.memset(c2sb[:, img, 0:16, 16], 0.0)

            for img in range(S):
                for h in range(2):
                    ps1 = pc_pool.tile([32, 16, 32], F32, tag="psc")
                    nc.tensor.matmul(
                        ps1[:].rearrange("p a b -> p (a b)"),
                        r32(w1_sb[:]),
                        r32(patches_sb[:, img, h * 512:(h + 1) * 512]),
                        start=True, stop=True)
                    # relu(x + bc1) -> padded layout; DVE (ACT is busier later)
                    out_ap = c1sb[:, img, h * 16:(h + 1) * 16, 0:32]
                    if img % 3 == 2:
                        nc.scalar.activation(out_ap, ps1[:], AF.Relu, bias=bc1_sb[:])
                    else:
                        nc.vector.tensor_scalar(out_ap, ps1[:], bc1_sb[:], 0.0,
                                                op0=OP.add, op1=OP.max)

            # ---- conv2: [32]->[48], 32x32 -> 16x16 ----
            for ip in range(3):      # image pairs
                ps2 = pc_pool.tile([48, 2, 16, 16], F32, tag="psc")
                for k, (dy, dx) in enumerate((dy, dx) for dy in range(3) for dx in range(3)):
                    nc.tensor.matmul(
                        ps2[:],
                        r32(w2_sb[:, k * 48:(k + 1) * 48]),
                        r32(c1sb[:, 2 * ip:2 * ip + 2, dy:dy + 31:2, dx:dx + 31:2]),
                        start=(k == 0), stop=(k == 8))
                out_ap = c2sb[:, 2 * ip:2 * ip + 2, 0:16, 0:16]
                if ip % 2 == 0:
                    nc.scalar.activation(out_ap, ps2[:], AF.Relu, bias=bc2_sb[:])
                else:
                    nc.vector.tensor_scalar(out_ap, ps2[:], bc2_sb[:], 0.0,
                                            op0=OP.add, op1=OP.max)

            # ---- conv3: [48]->[64], 16x16 -> 8x8 ----
            ps3 = pc_pool.tile([64, S, D, D], F32, tag="psc")
            for k, (dy, dx) in enumerate((dy, dx) for dy in range(3) for dx in range(3)):
                nc.tensor.matmul(
                    ps3[:],
                    r32(w3_sb[:, k * 64:(k + 1) * 64]),
                    r32(c2sb[:, :, dy:dy + 15:2, dx:dx + 15:2]),
                    start=(k == 0), stop=(k == 8))
            nc.scalar.activation(featc[0:64, :].rearrange("p (i m) -> p i m", m=M),
                                 ps3[:].rearrange("p i a b -> p i (a b)"),
                                 AF.Relu, bias=bc3_sb[:])

            # ---- u / v ----
            psu = pc_pool.tile([H1, S * M], F32, tag="psc")
            psv = pc_pool.tile([H1, S * M], F32, tag="psc")
            nc.tensor.matmul(psu[:], r32(w1a_sb[:]), r32(featc[:]), start=True, stop=True)
            nc.tensor.matmul(psv[:], r32(w1b_sb[:]), r32(featc[:]), start=True, stop=True)
            u_f32 = wpool.tile([H1, S * M], F32)
            v_bf = wpool.tile([H1, S * M], BF16)
            v_f32 = wpool.tile([H1, S * M], F32)
            nc.scalar.activation(u_f32[:], psu[:], AF.Copy)
            nc.vector.tensor_scalar(v_bf[:], psv[:], bg1_sb[:], None, op0=OP.add)
            nc.vector.tensor_scalar(v_f32[:], psv[:], bg1_sb[:], None, op0=OP.add)

            # ---- cls head (overlaps relation; DMA result out early) ----
            fme = wpool.tile([65, S], F32)
            nc.gpsimd.memset(fme[:], 1.0)
            nc.vector.tensor_reduce(
                fme[0:64, :], featc[0:64, :].rearrange("p (i m) -> p i m", m=M),
                axis=mybir.AxisListType.X, op=OP.add)
            psl = pc_pool.tile([S, NCls], F32, tag="psc")
            nc.tensor.matmul(psl[:], r32(fme[:]), r32(wle_sb[:]), start=True, stop=True)
            mx = wpool.tile([S, 1], F32)
            nc.vector.tensor_reduce(mx[:], psl[:], axis=mybir.AxisListType.X, op=OP.max)
            shifted = wpool.tile([S, NCls], F32)
            nc.vector.tensor_scalar(shifted[:], psl[:], mx[:], None, op0=OP.subtract)
            escr = wpool.tile([S, NCls], F32)
            se = wpool.tile([S, 1], F32)
            nc.scalar.activation(escr[:], shifted[:], AF.Exp, accum_out=se[:])
            selscr = wpool.tile([S, NCls], F32)
            sel = wpool.tile([S, 1], F32)
            nc.vector.tensor_tensor(selscr[:], shifted[:], onehot_sb[:], op=OP.mult)
            nc.vector.tensor_reduce(sel[:], selscr[:], axis=mybir.AxisListType.X, op=OP.add)
            nc.sync.dma_start(out=out_cls[:, 0:1], in_=se[:])
            nc.sync.dma_start(out=out_cls[:, 1:2], in_=sel[:])

            # ---- relation stage ----
            # xf_cols[:, (qh*9 + jl*3 + duo) + 18*gh] accumulates one evac op's
            # sum over (16q x 64p); combine gh then qh afterwards.
            xf_cols = wpool.tile([2 * CO, 36], F32)

            # engine assignment pattern for the 32 hdd-gen q ops of each unit
            gps_slots = set(int(i * 32 / KGPS) for i in range(KGPS)) if KGPS else set()
            rest = [s for s in range(32) if s not in gps_slots]
            act_slots = set(rest[int((i + 0.5) * len(rest) / KACT)]
                            for i in range(KACT)) if KACT else set()
            dve_evac_slots = (set(int((i + 0.5) * 6 / KDVE_EVAC)
                                  for i in range(KDVE_EVAC))
                              if KDVE_EVAC else set())

            for jl in range(3):
                for qh in range(2):
                    hdd = hpool.tile([H1, 32, S * M], BF16, tag="hdd")
                    for ql in range(32):
                        q = qh * 32 + ql
                        ucol = u_f32[:, jl * M + q: jl * M + q + 1]
                        if ql in act_slots:
                            nc.scalar.activation(hdd[:, ql, :], v_f32[:],
                                                 AF.Relu, bias=ucol)
                        elif ql in gps_slots:
                            nc.gpsimd.tensor_scalar(hdd[:, ql, :], v_f32[:],
                                                    ucol, 0.0,
                                                    op0=OP.add, op1=OP.max)
                        else:
                            nc.vector.tensor_scalar(hdd[:, ql, :], v_bf[:],
                                                    ucol, 0.0,
                                                    op0=OP.add, op1=OP.max)
                    for duo in range(3):
                        iA, iB = 2 * duo, 2 * duo + 1
                        for gh in range(2):
                            ps = pb_pool.tile([2 * CO, 1024], F32, tag="gps")
                            for q2 in range(2):
                                qg = gh * 2 + q2
                                nc.tensor.matmul(
                                    ps[0:CO, q2 * 512:(q2 + 1) * 512],
                                    wg2_sb[:],
                                    hdd[:, qg * 8:(qg + 1) * 8, iA * M:(iA + 1) * M],
                                    start=True, stop=True)
                                nc.tensor.matmul(
                                    ps[CO:2 * CO, q2 * 512:(q2 + 1) * 512],
                                    wg2_sb[:],
                                    hdd[:, qg * 8:(qg + 1) * 8, iB * M:(iB + 1) * M],
                                    start=True, stop=True,
                                    tile_position=(0, 64))
                            ucol_i = (qh * 9 + jl * 3 + duo) + 18 * gh
                            gscr = spool.tile([2 * CO, 1024], BF16, tag="gscr")
                            if (duo * 2 + gh) in dve_evac_slots:
                                nc.vector._custom_dve(
                                    RELU_BIAS_SUM, out=gscr[:], in0=ps[:],
                                    s0=bg2_sb[:],
                                    accum_out=xf_cols[:, ucol_i:ucol_i + 1])
                            else:
                                nc.scalar.activation(
                                    gscr[:], ps[:], AF.Relu,
                                    bias=bg2_sb[:],
                                    accum_out=xf_cols[:, ucol_i:ucol_i + 1])

            # ---- score head ----
            # sum the two gh-halves, then the two qh-halves
            xf18 = wpool.tile([2 * CO, 18], F32)
            nc.vector.tensor_tensor(
                xf18[:], xf_cols[:, 0:18], xf_cols[:, 18:36], op=OP.add)
            xf_pair = wpool.tile([2 * CO, 9], F32)
            nc.vector.tensor_tensor(
                xf_pair[:], xf18[:, 0:9], xf18[:, 9:18], op=OP.add)
            # assemble xf for both partition halves into one base-0 tile:
            # even i (rows 0:64) -> cols 0:9, odd i -> cols 9:18 (SBUF DMA)
            xf_ext = wpool.tile([CO, 18], F32)
            nc.vector.tensor_copy(xf_ext[:, 0:9], xf_pair[0:CO, :])
            nc.sync.dma_start(out=xf_ext[:, 9:18], in_=xf_pair[CO:2 * CO, :])
            psh1 = pc_pool.tile([16, 18], F32, tag="psc")
            nc.tensor.matmul(psh1[:], r32(wf1d_sb[0:CO, :]),
                             r32(xf_ext[:]), start=True, stop=True)
            h1e = wpool.tile([17, 18], F32)
            nc.gpsimd.memset(h1e[:], 1.0)
            nc.scalar.activation(h1e[0:16, :], psh1[:], AF.Relu, bias=bf1_sb[:])
            psh2 = pc_pool.tile([18, 1], F32, tag="psc")
            nc.tensor.matmul(psh2[:], r32(h1e[:]), r32(wf2e_sb[:]),
                             start=True, stop=True)
            en = wpool.tile([18, 1], F32)
            nc.scalar.activation(en[:], psh2[:], AF.Exp, scale=-1.0)
            ep1 = wpool.tile([18, 1], F32)
            nc.vector.tensor_scalar(ep1[:], en[:], 1.0, None, op0=OP.add)
            sc = wpool.tile([18, 1], F32)
            nc.vector.reciprocal(sc[:], ep1[:])
            nc.sync.dma_start(out=out_scores[:], in_=sc[:])
    nc.compile()
    return nc


_NC_CACHE = None


def _get_nc():
    global _NC_CACHE
    if _NC_CACHE is None:
        _NC_CACHE = _build_nc()
    return _NC_CACHE


def _host_prep(inputs):
    ins = {k: np.asarray(v) for k, v in inputs.items()}
    x = np.concatenate([ins['support_x'], ins['query_x']], axis=1)
    lab = np.concatenate([ins['support_y'], ins['query_y']], axis=1)

    xpad = np.pad(x.astype(np.float32), ((0, 0), (0, 0), (0, 0), (0, 1), (0, 1)))
    win = np.lib.stride_tricks.sliding_window_view(xpad, (3, 3), axis=(3, 4))
    win = win[:, :, :, ::2, ::2]
    patches = win.transpose(0, 2, 5, 6, 1, 3, 4).reshape(B, 27, S, 1024)
    patches = np.ascontiguousarray(patches, np.float32)

    f32 = np.float32
    bf16 = ml_dtypes.bfloat16
    w1 = np.ascontiguousarray(ins['k1'].reshape(32, 27).T, f32).astype(bf16)
    w2 = np.ascontiguousarray(ins['k2'].transpose(1, 2, 3, 0).reshape(32, 9 * 48), f32).astype(bf16)
    w3 = np.ascontiguousarray(ins['k3'].transpose(1, 2, 3, 0).reshape(48, 9 * 64), f32).astype(bf16)

    ii = np.arange(D, dtype=f32) / D
    coord = np.stack([np.broadcast_to(ii[:, None], (D, D)),
                      np.broadcast_to(ii[None, :], (D, D))]).reshape(2, M)
    coords = np.ascontiguousarray(np.tile(coord, (1, S)), f32).astype(bf16)

    onehots = np.zeros((B, S, NCls), f32)
    for b in range(B):
        onehots[b, np.arange(S), lab[b]] = 1.0

    Wg1 = ins['Wg1'].astype(f32)
    common = dict(
        w1=w1, w2=w2, w3=w3,
        bc1=np.ascontiguousarray(ins['bc1'].reshape(32, 1), f32),
        bc2=np.ascontiguousarray(ins['bc2'].reshape(48, 1), f32),
        bc3=np.ascontiguousarray(ins['bc3'].reshape(64, 1), f32),
        coords=coords,
        wle=np.ascontiguousarray(
            np.vstack([ins['Wlog'].astype(f32) / M, ins['blog'][None, :].astype(f32)])),
        w1a=np.ascontiguousarray(Wg1[:C2]).astype(bf16),
        w1b=np.ascontiguousarray(Wg1[C2:]).astype(bf16),
        bg1=np.ascontiguousarray(ins['bg1'].reshape(H1, 1), f32),
        wg2=np.ascontiguousarray(ins['Wg2'], f32).astype(ml_dtypes.bfloat16),
        bg2_2=np.ascontiguousarray(np.tile(ins['bg2'].astype(f32), 2).reshape(2 * CO, 1)),
        wf1d=np.ascontiguousarray(
            np.vstack([ins['Wf1'].astype(f32), ins['Wf1'].astype(f32)])),
        bf1=np.ascontiguousarray(ins['bf1'].reshape(16, 1), f32),
        wf2e=np.ascontiguousarray(
            np.vstack([ins['Wf2'].astype(f32), ins['bf2'].reshape(1, 1).astype(f32)])),
    )
    in_maps = []
    for core in range(N_CORES):
        b, half = core // 2, core % 2
        # odd cores see images in rotated order so the program's local
        # j in {0,1,2} maps to global j in {3,4,5}
        perm = (0, 1, 2, 3, 4, 5) if half == 0 else (3, 4, 5, 0, 1, 2)
        m = dict(common)
        m['patches'] = np.ascontiguousarray(patches[b][:, perm, :]).astype(ml_dtypes.bfloat16)
        m['onehot'] = np.ascontiguousarray(onehots[b][list(perm)])
        in_maps.append(m)
    return in_maps, lab


def _host_post(results, lab):
    P = np.zeros((B, S, S), np.float32)
    cls_terms = np.zeros((B, S), np.float32)
    for core in range(N_CORES):
        b, half = core // 2, core % 2
        perm = (0, 1, 2, 3, 4, 5) if half == 0 else (3, 4, 5, 0, 1, 2)
        sc = results[core]["scores"].reshape(18)
        # score col layout: k < 9: (jl = k//3, duo = k%3, i = 2*duo);
        #                   k >= 9: i = 2*duo + 1
        for k in range(18):
            kk = k % 9
            jl, duo = kk // 3, kk % 3
            i = 2 * duo + (1 if k >= 9 else 0)
            P[b, perm[i], perm[jl]] = sc[k]
        if half == 0:
            sesel = results[core]["clsv"].reshape(S, 2)
            cls_terms[b] = np.log(sesel[:, 0]) - sesel[:, 1]
    cls_loss = np.float32(cls_terms.mean())
    y = (lab[:, :, None] == lab[:, None, :]).astype(np.float32)
    Pt = P.transpose(0, 2, 1)
    sym, anti = np.float32(0.5) * (P + Pt), np.float32(0.5) * (P - Pt)
    sym_n = np.sqrt((sym ** 2).sum(axis=(1, 2)))
    anti_n = np.sqrt((anti ** 2).sum(axis=(1, 2)))
    sym_loss = np.float32(((sym_n - anti_n) / (sym_n + anti_n)).mean())
    euc_loss = np.float32(((P - y) ** 2).mean())
    rn_loss = np.float32(euc_loss - np.float32(0.1) * sym_loss)
    return np.float32(cls_loss), np.float32(rn_loss), np.float32(sym_loss)


def run_spmd(inputs, trace=False, **kwargs):
    nc = _get_nc()
    in_maps, lab = _host_prep(inputs)
    res = run_bass_kernel_spmd(nc, in_maps, list(range(N_CORES)),
                               trace=trace, **kwargs)
    return _host_post(res.results, lab), res


def kernel(**inputs):
    out, _ = run_spmd(inputs)
    return out


# revision 31
# speedup vs baseline: 4.3838x; 1.0275x over previous
"""Trainium2 Bass kernel for nn_Meta_67078799229377 (relation-network meta-learner).

Sharding: 8 cores = 4 batch elements x 2 halves of the relation-j axis.
Each core runs the full backbone for its batch element's 6 images, then the
relation network for its 18 (i, j) pairs, fully fused on-chip (the
[s,s,m,m,128] tensor never exists in HBM). Host code only reshapes/shards
inputs and combines 144 scores + 24 per-sample CE terms into the 3 scalar
losses.

v2 changes vs v1:
 - input DMAs reordered (patches first) and spread across engine queues
 - consolidated pad memsets, early ACT table prefetch (exp/ln)
 - hdd generation split across DVE/ACT/GPSIMD (env-tunable)
 - g evacuation in FD=2048 ops split ACT/DVE, PSUM tiles [128,2048]
 - score head uses two matmuls on xf partition halves (no SBUF-shift DMA)
 - cls output DMA issued as soon as ready
"""
import os
import numpy as np
import ml_dtypes

import concourse.bass as bass
import concourse.mybir as mybir
import concourse.tile as tile
from concourse import bacc
from concourse.bass_utils import run_bass_kernel_spmd

F32 = mybir.dt.float32
BF16 = mybir.dt.bfloat16
AF = mybir.ActivationFunctionType
OP = mybir.AluOpType


def _register_relu_bias_sum():
    """Custom DVE op: out = relu(in0 + s0), accum_out = sum(out).

    Fuses the g-evacuation (relu + bias + spatial-sum) into one Vector-engine
    instruction; stock tensor_scalar repurposes op1 as the reduce op when
    accum_out is attached, so it cannot express this.
    """
    from concourse import dve_ops
    from concourse.dve_spec import Spec, Src0, C0, Zero, relu, lower
    from concourse.dve_spec import _has_src1 as has_src1
    from concourse.dve_uop import DveOpSpec
    from operator import add as op_add

    name = "RELU_BIAS_SUM_ANT"
    for op in dve_ops.OPS:
        if op.name == name:
            return op

    def _ref(in0, in1, c0, c1, c2):
        b = np.maximum(in0.astype(np.float32) + c0, 0).astype(np.float32)
        return b, b.reshape(b.shape[0], -1).sum(axis=-1, keepdims=True)

    row = max(dve_ops._SUB_OPCODE_FOR_NAME.values()) + 1
    assert row < 0x20
    dve_ops._SUB_OPCODE_FOR_NAME[name] = row
    spec = Spec(body=relu(Src0 + C0), accum=op_add, accum_init=Zero,
                reference=_ref)
    shas = {}
    for ver in ("v3", "v4"):
        dspec = DveOpSpec(name=name, opcode=row, uops=lower(spec, ver=ver),
                          rd1_en=has_src1(spec))
        shas[ver] = dspec.sha(ver)
    op = dve_ops.DveOp(name, spec, subdim=False, uops_sha=shas)
    dve_ops.OPS.append(op)
    dve_ops.CUSTOM_DVE_SPECS[name] = spec
    return op


RELU_BIAS_SUM = None
if int(os.environ.get("KDVE_EVAC", "0")) > 0:
    RELU_BIAS_SUM = _register_relu_bias_sum()

B, S, D = 4, 6, 8
M = D * D            # 64 spatial positions
C2 = 66              # 64 channels + 2 coord channels
H1 = 128             # g-MLP hidden
CO = 64              # g-MLP out
NCls = 64
N_CORES = 8

# hdd-gen engine split per unit (32 q-ops): gpsimd + act counts; rest on DVE.
KGPS = int(os.environ.get("KGPS", "0"))
KACT = int(os.environ.get("KACT", "1"))
# of the 6 per-unit evacuation ops, how many go to DVE (rest on ACT)
KDVE_EVAC = int(os.environ.get("KDVE_EVAC", "0"))
# which engines take non-sync const DMAs: "sync" | "spread"
KDMA = os.environ.get("KDMA", "sync")


def _build_nc():
    nc = bacc.Bacc("TRN2", target_bir_lowering=False, debug=False,
                   num_devices=N_CORES)

    din = {}
    def dram_in(name, shape, dtype=F32):
        din[name] = nc.dram_tensor(name, list(shape), dtype, kind="ExternalInput")
        return din[name]

    # patches4: image j lives at partition base 32*(j%4), slot j//4, so four
    # conv1 matmuls can run concurrently in distinct PE row groups.
    x_patches = dram_in("patches", [128, 2, 1024], BF16)
    x_w1 = dram_in("w1", [128, 32], BF16)
    x_w2 = dram_in("w2", [32, 9 * 48], BF16)
    x_w3 = dram_in("w3", [48, 9 * 64], BF16)
    x_bc1 = dram_in("bc1", [32, 1])
    x_bc2 = dram_in("bc2", [48, 1])
    x_bc3 = dram_in("bc3", [64, 1])
    x_coords = dram_in("coords", [2, S * M], BF16)
    x_wle = dram_in("wle", [65, NCls])
    x_onehot = dram_in("onehot", [S, NCls])
    x_w1a = dram_in("w1a", [C2, H1], BF16)
    x_w1b = dram_in("w1b", [C2, H1], BF16)
    x_bg1 = dram_in("bg1", [H1, 1])
    x_wg2 = dram_in("wg2", [H1, CO], BF16)
    x_bg2 = dram_in("bg2_2", [2 * CO, 1])
    x_wf1d = dram_in("wf1d", [2 * CO, 16])   # Wf1 stacked twice (for row halves)
    x_bf1 = dram_in("bf1", [16, 1])
    x_wf2e = dram_in("wf2e", [17, 1])

    out_scores = nc.dram_tensor("scores", [18, 1], F32, kind="ExternalOutput")
    # clsv[:, 0] = sum(exp(shifted)); clsv[:, 1] = selected shifted logit.
    # Host computes ln(se) - sel, so the device never needs the Ln table.
    out_cls = nc.dram_tensor("clsv", [S, 2], F32, kind="ExternalOutput")

    with tile.TileContext(nc) as tc:
        with (
            tc.tile_pool(name="const", bufs=1) as cpool,
            tc.tile_pool(name="work", bufs=1) as wpool,
            tc.tile_pool(name="patch", bufs=1) as ppool,
            tc.tile_pool(name="hdd", bufs=4) as hpool,
            tc.tile_pool(name="gscr", bufs=3) as spool,
            tc.tile_pool(name="psmall", bufs=2, space="PSUM") as pc_pool,
            tc.tile_pool(name="pbig", bufs=3, space="PSUM") as pb_pool,
        ):
            # ---- inputs to SBUF; order matters: conv-critical first on sync ----
            def c_tile(src, shape, dtype=F32, eng=None):
                t = cpool.tile(list(shape), dtype, tag=src.name)
                (eng or nc.sync).dma_start(out=t[:], in_=src[:])
                return t

            patches_sb = ppool.tile([128, 2, 1024], BF16)
            nc.sync.dma_start(out=patches_sb[:], in_=x_patches[:])
            w1_sb = c_tile(x_w1, [128, 32], BF16)
            bc1_sb = c_tile(x_bc1, [32, 1])
            w2_sb = c_tile(x_w2, [32, 9 * 48], BF16)
            bc2_sb = c_tile(x_bc2, [48, 1])
            w3_sb = c_tile(x_w3, [48, 9 * 64], BF16)
            bc3_sb = c_tile(x_bc3, [64, 1])

            featc = wpool.tile([C2, S * M], BF16)
            # non-critical consts; optionally spread across the scalar queue
            # (only SP/Activation/gpsimd can initiate DMAs)
            alt = nc.scalar if KDMA == "spread" else nc.sync
            nc.sync.dma_start(out=featc[64:66, :], in_=x_coords[:])
            w1a_sb = c_tile(x_w1a, [C2, H1], BF16, eng=alt)
            w1b_sb = c_tile(x_w1b, [C2, H1], BF16, eng=alt)
            bg1_sb = c_tile(x_bg1, [H1, 1], eng=alt)
            wg2_sb = c_tile(x_wg2, [H1, CO], BF16, eng=alt)
            bg2_sb = c_tile(x_bg2, [2 * CO, 1], eng=alt)
            wle_sb = c_tile(x_wle, [65, NCls])
            onehot_sb = c_tile(x_onehot, [S, NCls])
            wf1d_sb = c_tile(x_wf1d, [2 * CO, 16])
            bf1_sb = c_tile(x_bf1, [16, 1])
            wf2e_sb = c_tile(x_wf2e, [17, 1])

            def r32(ap):
                return ap

            # ---- conv1: [27]->[32], 64x64 -> 32x32 (stride 2, im2col'd) ----
            c1sb = wpool.tile([32, S, 33, 33], BF16)
            c2sb = wpool.tile([48, S, 17, 17], BF16)
            for img in range(S):
                nc.gpsimd.memset(c1sb[:, img, 32, :], 0.0)
                nc.gpsimd.memset(c1sb[:, img, 0:32, 32], 0.0)
                nc.gpsimd.memset(c2sb[:, img, 16, :], 0.0)
                nc.gpsimd.memset(c2sb[:, img, 0:16, 16], 0.0)

            for h in range(2):
                for grp in ((0, 1, 2), (3, 4, 5)):
                    tiles = []
                    for img in grp:
                        pb = 32 * (img % 4)
                        slot = img // 4
                        ps1 = pb_pool.tile([32, 16, 32], F32, tag="gps")
                        nc.tensor.matmul(
                            ps1[:].rearrange("p a b -> p (a b)"),
                            r32(w1_sb[pb:pb + 27, :]),
                            r32(patches_sb[pb:pb + 27, slot,
                                           h * 512:(h + 1) * 512]),
                            start=True, stop=True, tile_position=(pb, 0))
                        tiles.append((img, ps1))
                    for img, ps1 in tiles:
                        # relu(x + bc1) -> padded layout
                        out_ap = c1sb[:, img, h * 16:(h + 1) * 16, 0:32]
                        if img % 3 == 2:
                            nc.scalar.activation(out_ap, ps1[:], AF.Relu,
                                                 bias=bc1_sb[:])
                        else:
                            nc.vector.tensor_scalar(out_ap, ps1[:], bc1_sb[:],
                                                    0.0, op0=OP.add, op1=OP.max)

            # ---- conv2: [32]->[48], 32x32 -> 16x16 ----
            for ip in range(3):      # image pairs
                ps2 = pc_pool.tile([48, 2, 16, 16], F32, tag="psc")
                for k, (dy, dx) in enumerate((dy, dx) for dy in range(3) for dx in range(3)):
                    nc.tensor.matmul(
                        ps2[:],
                        r32(w2_sb[:, k * 48:(k + 1) * 48]),
                        r32(c1sb[:, 2 * ip:2 * ip + 2, dy:dy + 31:2, dx:dx + 31:2]),
                        start=(k == 0), stop=(k == 8))
                out_ap = c2sb[:, 2 * ip:2 * ip + 2, 0:16, 0:16]
                if ip % 2 == 0:
                    nc.scalar.activation(out_ap, ps2[:], AF.Relu, bias=bc2_sb[:])
                else:
                    nc.vector.tensor_scalar(out_ap, ps2[:], bc2_sb[:], 0.0,
                                            op0=OP.add, op1=OP.max)

            # ---- conv3: [48]->[64], 16x16 -> 8x8 ----
            ps3 = pc_pool.tile([64, S, D, D], F32, tag="psc")
            for k, (dy, dx) in enumerate((dy, dx) for dy in range(3) for dx in range(3)):
                nc.tensor.matmul(
                    ps3[:],
                    r32(w3_sb[:, k * 64:(k + 1) * 64]),
                    r32(c2sb[:, :, dy:dy + 15:2, dx:dx + 15:2]),
                    start=(k == 0), stop=(k == 8))
            nc.scalar.activation(featc[0:64, :].rearrange("p (i m) -> p i m", m=M),
                                 ps3[:].rearrange("p i a b -> p i (a b)"),
                                 AF.Relu, bias=bc3_sb[:])

            # ---- u / v ----
            psu = pc_pool.tile([H1, S * M], F32, tag="psc")
            psv = pc_pool.tile([H1, S * M], F32, tag="psc")
            nc.tensor.matmul(psu[:], r32(w1a_sb[:]), r32(featc[:]), start=True, stop=True)
            nc.tensor.matmul(psv[:], r32(w1b_sb[:]), r32(featc[:]), start=True, stop=True)
            u_f32 = wpool.tile([H1, S * M], F32)
            v_bf = wpool.tile([H1, S * M], BF16)
            v_f32 = wpool.tile([H1, S * M], F32)
            nc.scalar.activation(u_f32[:], psu[:], AF.Copy)
            nc.vector.tensor_scalar(v_bf[:], psv[:], bg1_sb[:], None, op0=OP.add)
            nc.vector.tensor_scalar(v_f32[:], psv[:], bg1_sb[:], None, op0=OP.add)

            # ---- cls head (overlaps relation; DMA result out early) ----
            fme = wpool.tile([65, S], F32)
            nc.gpsimd.memset(fme[:], 1.0)
            nc.vector.tensor_reduce(
                fme[0:64, :], featc[0:64, :].rearrange("p (i m) -> p i m", m=M),
                axis=mybir.AxisListType.X, op=OP.add)
            psl = pc_pool.tile([S, NCls], F32, tag="psc")
            nc.tensor.matmul(psl[:], r32(fme[:]), r32(wle_sb[:]), start=True, stop=True)
            mx = wpool.tile([S, 1], F32)
            nc.vector.tensor_reduce(mx[:], psl[:], axis=mybir.AxisListType.X, op=OP.max)
            shifted = wpool.tile([S, NCls], F32)
            nc.vector.tensor_scalar(shifted[:], psl[:], mx[:], None, op0=OP.subtract)
            escr = wpool.tile([S, NCls], F32)
            se = wpool.tile([S, 1], F32)
            nc.scalar.activation(escr[:], shifted[:], AF.Exp, accum_out=se[:])
            selscr = wpool.tile([S, NCls], F32)
            sel = wpool.tile([S, 1], F32)
            nc.vector.tensor_tensor(selscr[:], shifted[:], onehot_sb[:], op=OP.mult)
            nc.vector.tensor_reduce(sel[:], selscr[:], axis=mybir.AxisListType.X, op=OP.add)
            nc.sync.dma_start(out=out_cls[:, 0:1], in_=se[:])
            nc.sync.dma_start(out=out_cls[:, 1:2], in_=sel[:])

            # ---- relation stage ----
            # xf_cols[:, (qh*9 + jl*3 + duo) + 18*gh] accumulates one evac op's
            # sum over (16q x 64p); combine gh then qh afterwards.
            xf_cols = wpool.tile([2 * CO, 36], F32)

            # engine assignment pattern for the 32 hdd-gen q ops of each unit
            gps_slots = set(int(i * 32 / KGPS) for i in range(KGPS)) if KGPS else set()
            rest = [s for s in range(32) if s not in gps_slots]
            act_slots = set(rest[int((i + 0.5) * len(rest) / KACT)]
                            for i in range(KACT)) if KACT else set()
            dve_evac_slots = (set(int((i + 0.5) * 6 / KDVE_EVAC)
                                  for i in range(KDVE_EVAC))
                              if KDVE_EVAC else set())

            for jl in range(3):
                for qh in range(2):
                    hdd = hpool.tile([H1, 32, S * M], BF16, tag="hdd")
                    for ql in range(32):
                        q = qh * 32 + ql
                        ucol = u_f32[:, jl * M + q: jl * M + q + 1]
                        if ql in act_slots:
                            nc.scalar.activation(hdd[:, ql, :], v_f32[:],
                                                 AF.Relu, bias=ucol)
                        elif ql in gps_slots:
                            nc.gpsimd.tensor_scalar(hdd[:, ql, :], v_f32[:],
                                                    ucol, 0.0,
                                                    op0=OP.add, op1=OP.max)
                        else:
                            nc.vector.tensor_scalar(hdd[:, ql, :], v_bf[:],
                                                    ucol, 0.0,
                                                    op0=OP.add, op1=OP.max)
                    for duo in range(3):
                        iA, iB = 2 * duo, 2 * duo + 1
                        for gh in range(2):
                            ps = pb_pool.tile([2 * CO, 1024], F32, tag="gps")
                            for q2 in range(2):
                                qg = gh * 2 + q2
                                nc.tensor.matmul(
                                    ps[0:CO, q2 * 512:(q2 + 1) * 512],
                                    wg2_sb[:],
                                    hdd[:, qg * 8:(qg + 1) * 8, iA * M:(iA + 1) * M],
                                    start=True, stop=True)
                                nc.tensor.matmul(
                                    ps[CO:2 * CO, q2 * 512:(q2 + 1) * 512],
                                    wg2_sb[:],
                                    hdd[:, qg * 8:(qg + 1) * 8, iB * M:(iB + 1) * M],
                                    start=True, stop=True,
                                    tile_position=(0, 64))
                            ucol_i = (qh * 9 + jl * 3 + duo) + 18 * gh
                            gscr = spool.tile([2 * CO, 1024], BF16, tag="gscr")
                            if (duo * 2 + gh) in dve_evac_slots:
                                nc.vector._custom_dve(
                                    RELU_BIAS_SUM, out=gscr[:], in0=ps[:],
                                    s0=bg2_sb[:],
                                    accum_out=xf_cols[:, ucol_i:ucol_i + 1])
                            else:
                                nc.scalar.activation(
                                    gscr[:], ps[:], AF.Relu,
                                    bias=bg2_sb[:],
                                    accum_out=xf_cols[:, ucol_i:ucol_i + 1])

            # ---- score head ----
            # sum the two gh-halves, then the two qh-halves
            xf18 = wpool.tile([2 * CO, 18], F32)
            nc.vector.tensor_tensor(
                xf18[:], xf_cols[:, 0:18], xf_cols[:, 18:36], op=OP.add)
            xf_pair = wpool.tile([2 * CO, 9], F32)
            nc.vector.tensor_tensor(
                xf_pair[:], xf18[:, 0:9], xf18[:, 9:18], op=OP.add)
            # assemble xf for both partition halves into one base-0 tile:
            # even i (rows 0:64) -> cols 0:9, odd i -> cols 9:18 (SBUF DMA)
            xf_ext = wpool.tile([CO, 18], F32)
            nc.vector.tensor_copy(xf_ext[:, 0:9], xf_pair[0:CO, :])
            nc.sync.dma_start(out=xf_ext[:, 9:18], in_=xf_pair[CO:2 * CO, :])
            psh1 = pc_pool.tile([16, 18], F32, tag="psc")
            nc.tensor.matmul(psh1[:], r32(wf1d_sb[0:CO, :]),
                             r32(xf_ext[:]), start=True, stop=True)
            h1e = wpool.tile([17, 18], F32)
            nc.gpsimd.memset(h1e[:], 1.0)
            nc.scalar.activation(h1e[0:16, :], psh1[:], AF.Relu, bias=bf1_sb[:])
            psh2 = pc_pool.tile([18, 1], F32, tag="psc")
            nc.tensor.matmul(psh2[:], r32(h1e[:]), r32(wf2e_sb[:]),
                             start=True, stop=True)
            en = wpool.tile([18, 1], F32)
            nc.scalar.activation(en[:], psh2[:], AF.Exp, scale=-1.0)
            ep1 = wpool.tile([18, 1], F32)
            nc.vector.tensor_scalar(ep1[:], en[:], 1.0, None, op0=OP.add)
            sc = wpool.tile([18, 1], F32)
            nc.vector.reciprocal(sc[:], ep1[:])
            nc.sync.dma_start(out=out_scores[:], in_=sc[:])
    nc.compile()
    return nc


_NC_CACHE = None


def _get_nc():
    global _NC_CACHE
    if _NC_CACHE is None:
        _NC_CACHE = _build_nc()
    return _NC_CACHE


def _host_prep(inputs):
    ins = {k: np.asarray(v) for k, v in inputs.items()}
    x = np.concatenate([ins['support_x'], ins['query_x']], axis=1)
    lab = np.concatenate([ins['support_y'], ins['query_y']], axis=1)

    xpad = np.pad(x.astype(np.float32), ((0, 0), (0, 0), (0, 0), (0, 1), (0, 1)))
    win = np.lib.stride_tricks.sliding_window_view(xpad, (3, 3), axis=(3, 4))
    win = win[:, :, :, ::2, ::2]
    patches = win.transpose(0, 2, 5, 6, 1, 3, 4).reshape(B, 27, S, 1024)
    patches = np.ascontiguousarray(patches, np.float32)

    f32 = np.float32
    bf16 = ml_dtypes.bfloat16
    w1s = np.ascontiguousarray(ins['k1'].reshape(32, 27).T, f32)
    w1 = np.zeros((128, 32), f32)
    for k in range(4):
        w1[32 * k:32 * k + 27, :] = w1s
    w1 = w1.astype(bf16)
    w2 = np.ascontiguousarray(ins['k2'].transpose(1, 2, 3, 0).reshape(32, 9 * 48), f32).astype(bf16)
    w3 = np.ascontiguousarray(ins['k3'].transpose(1, 2, 3, 0).reshape(48, 9 * 64), f32).astype(bf16)

    ii = np.arange(D, dtype=f32) / D
    coord = np.stack([np.broadcast_to(ii[:, None], (D, D)),
                      np.broadcast_to(ii[None, :], (D, D))]).reshape(2, M)
    coords = np.ascontiguousarray(np.tile(coord, (1, S)), f32).astype(bf16)

    onehots = np.zeros((B, S, NCls), f32)
    for b in range(B):
        onehots[b, np.arange(S), lab[b]] = 1.0

    Wg1 = ins['Wg1'].astype(f32)
    common = dict(
        w1=w1, w2=w2, w3=w3,
        bc1=np.ascontiguousarray(ins['bc1'].reshape(32, 1), f32),
        bc2=np.ascontiguousarray(ins['bc2'].reshape(48, 1), f32),
        bc3=np.ascontiguousarray(ins['bc3'].reshape(64, 1), f32),
        coords=coords,
        wle=np.ascontiguousarray(
            np.vstack([ins['Wlog'].astype(f32) / M, ins['blog'][None, :].astype(f32)])),
        w1a=np.ascontiguousarray(Wg1[:C2]).astype(bf16),
        w1b=np.ascontiguousarray(Wg1[C2:]).astype(bf16),
        bg1=np.ascontiguousarray(ins['bg1'].reshape(H1, 1), f32),
        wg2=np.ascontiguousarray(ins['Wg2'], f32).astype(ml_dtypes.bfloat16),
        bg2_2=np.ascontiguousarray(np.tile(ins['bg2'].astype(f32), 2).reshape(2 * CO, 1)),
        wf1d=np.ascontiguousarray(
            np.vstack([ins['Wf1'].astype(f32), ins['Wf1'].astype(f32)])),
        bf1=np.ascontiguousarray(ins['bf1'].reshape(16, 1), f32),
        wf2e=np.ascontiguousarray(
            np.vstack([ins['Wf2'].astype(f32), ins['bf2'].reshape(1, 1).astype(f32)])),
    )
    in_maps = []
    for core in range(N_CORES):
        b, half = core // 2, core % 2
        # odd cores see images in rotated order so the program's local
        # j in {0,1,2} maps to global j in {3,4,5}
        perm = (0, 1, 2, 3, 4, 5) if half == 0 else (3, 4, 5, 0, 1, 2)
        m = dict(common)
        p4 = np.zeros((128, 2, 1024), f32)
        for j in range(S):
            p4[32 * (j % 4):32 * (j % 4) + 27, j // 4, :] = patches[b][:, perm[j], :]
        m['patches'] = p4.astype(ml_dtypes.bfloat16)
        m['onehot'] = np.ascontiguousarray(onehots[b][list(perm)])
        in_maps.append(m)
    return in_maps, lab


def _host_post(results, lab):
    P = np.zeros((B, S, S), np.float32)
    cls_terms = np.zeros((B, S), np.float32)
    for core in range(N_CORES):
        b, half = core // 2, core % 2
        perm = (0, 1, 2, 3, 4, 5) if half == 0 else (3, 4, 5, 0, 1, 2)
        sc = results[core]["scores"].reshape(18)
        # score col layout: k < 9: (jl = k//3, duo = k%3, i = 2*duo);
        #                   k >= 9: i = 2*duo + 1
        for k in range(18):
            kk = k % 9
            jl, duo = kk // 3, kk % 3
            i = 2 * duo + (1 if k >= 9 else 0)
            P[b, perm[i], perm[jl]] = sc[k]
        if half == 0:
            sesel = results[core]["clsv"].reshape(S, 2)
            cls_terms[b] = np.log(sesel[:, 0]) - sesel[:, 1]
    cls_loss = np.float32(cls_terms.mean())
    y = (lab[:, :, None] == lab[:, None, :]).astype(np.float32)
    Pt = P.transpose(0, 2, 1)
    sym, anti = np.float32(0.5) * (P + Pt), np.float32(0.5) * (P - Pt)
    sym_n = np.sqrt((sym ** 2).sum(axis=(1, 2)))
    anti_n = np.sqrt((anti ** 2).sum(axis=(1, 2)))
    sym_loss = np.float32(((sym_n - anti_n) / (sym_n + anti_n)).mean())
    euc_loss = np.float32(((P - y) ** 2).mean())
    rn_loss = np.float32(euc_loss - np.float32(0.1) * sym_loss)
    return np.float32(cls_loss), np.float32(rn_loss), np.float32(sym_loss)


def run_spmd(inputs, trace=False, **kwargs):
    nc = _get_nc()
    in_maps, lab = _host_prep(inputs)
    res = run_bass_kernel_spmd(nc, in_maps, list(range(N_CORES)),
                               trace=trace, **kwargs)
    return _host_post(res.results, lab), res


def kernel(**inputs):
    out, _ = run_spmd(inputs)
    return out


# revision 40
# speedup vs baseline: 4.7762x; 1.0895x over previous
"""Trainium2 Bass kernel for nn_Meta_67078799229377 (relation-network meta-learner).

Sharding: 8 cores = 4 batch elements x 2 halves of the relation-j axis.
Each core runs the full backbone for its batch element's 6 images, then the
relation network for its 18 (i, j) pairs, fully fused on-chip (the
[s,s,m,m,128] tensor never exists in HBM). Host code only reshapes/shards
inputs and combines 144 scores + 24 per-sample CE terms into the 3 scalar
losses.

v2 changes vs v1:
 - input DMAs reordered (patches first) and spread across engine queues
 - consolidated pad memsets, early ACT table prefetch (exp/ln)
 - hdd generation split across DVE/ACT/GPSIMD (env-tunable)
 - g evacuation in FD=2048 ops split ACT/DVE, PSUM tiles [128,2048]
 - score head uses two matmuls on xf partition halves (no SBUF-shift DMA)
 - cls output DMA issued as soon as ready
"""
import os
import numpy as np
import ml_dtypes

import concourse.bass as bass
import concourse.mybir as mybir
import concourse.tile as tile
from concourse import bacc
from concourse.bass_utils import run_bass_kernel_spmd

F32 = mybir.dt.float32
BF16 = mybir.dt.bfloat16
AF = mybir.ActivationFunctionType
OP = mybir.AluOpType


def _register_relu_bias_sum():
    """Custom DVE op: out = relu(in0 + s0), accum_out = sum(out).

    Fuses the g-evacuation (relu + bias + spatial-sum) into one Vector-engine
    instruction; stock tensor_scalar repurposes op1 as the reduce op when
    accum_out is attached, so it cannot express this.
    """
    from concourse import dve_ops
    from concourse.dve_spec import Spec, Src0, C0, Zero, relu, lower
    from concourse.dve_spec import _has_src1 as has_src1
    from concourse.dve_uop import DveOpSpec
    from operator import add as op_add

    name = "RELU_BIAS_SUM_ANT"
    for op in dve_ops.OPS:
        if op.name == name:
            return op

    def _ref(in0, in1, c0, c1, c2):
        b = np.maximum(in0.astype(np.float32) + c0, 0).astype(np.float32)
        return b, b.reshape(b.shape[0], -1).sum(axis=-1, keepdims=True)

    row = max(dve_ops._SUB_OPCODE_FOR_NAME.values()) + 1
    assert row < 0x20
    dve_ops._SUB_OPCODE_FOR_NAME[name] = row
    spec = Spec(body=relu(Src0 + C0), accum=op_add, accum_init=Zero,
                reference=_ref)
    shas = {}
    for ver in ("v3", "v4"):
        dspec = DveOpSpec(name=name, opcode=row, uops=lower(spec, ver=ver),
                          rd1_en=has_src1(spec))
        shas[ver] = dspec.sha(ver)
    op = dve_ops.DveOp(name, spec, subdim=False, uops_sha=shas)
    dve_ops.OPS.append(op)
    dve_ops.CUSTOM_DVE_SPECS[name] = spec
    return op


RELU_BIAS_SUM = None
if int(os.environ.get("KDVE_EVAC", "0")) > 0:
    RELU_BIAS_SUM = _register_relu_bias_sum()

B, S, D = 4, 6, 8
M = D * D            # 64 spatial positions
C2 = 66              # 64 channels + 2 coord channels
H1 = 128             # g-MLP hidden
CO = 64              # g-MLP out
NCls = 64
N_CORES = 8

# hdd-gen engine split per unit (32 q-ops): gpsimd + act counts; rest on DVE.
KGPS = int(os.environ.get("KGPS", "0"))
KACT = int(os.environ.get("KACT", "1"))
# of the 6 per-unit evacuation ops, how many go to DVE (rest on ACT)
KDVE_EVAC = int(os.environ.get("KDVE_EVAC", "0"))
# which engines take non-sync const DMAs: "sync" | "spread"
KDMA = os.environ.get("KDMA", "sync")


def _build_nc():
    nc = bacc.Bacc("TRN2", target_bir_lowering=False, debug=False,
                   num_devices=N_CORES)

    din = {}
    def dram_in(name, shape, dtype=F32):
        din[name] = nc.dram_tensor(name, list(shape), dtype, kind="ExternalInput")
        return din[name]

    # patches4: image j lives at partition base 32*(j%4), slot j//4, so four
    # conv1 matmuls can run concurrently in distinct PE row groups.
    x_patches = dram_in("patches", [128, 2, 1024], BF16)
    x_w1 = dram_in("w1", [128, 32], BF16)
    x_w2 = dram_in("w2", [128, 9 * 48], BF16)   # 4 copies at bases 0/32/64/96
    x_w3 = dram_in("w3", [128, 9 * 64], BF16)   # 2 copies at bases 0/64
    x_bc1 = dram_in("bc1", [128, 1])    # 4 copies at bases 0/32/64/96
    x_bc2 = dram_in("bc2", [128, 1])    # 2 copies at bases 0/64
    x_bc3 = dram_in("bc3", [64, 1])
    x_coords = dram_in("coords", [2, S * M], BF16)
    x_wle = dram_in("wle", [65, NCls])
    x_onehot = dram_in("onehot", [S, NCls])
    x_w1a = dram_in("w1a", [C2, H1], BF16)
    x_w1b = dram_in("w1b", [C2, H1], BF16)
    x_bg1 = dram_in("bg1", [H1, 1])
    x_wg2 = dram_in("wg2", [H1, CO], BF16)
    x_bg2 = dram_in("bg2_2", [2 * CO, 1])
    x_wf1d = dram_in("wf1d", [2 * CO, 16])   # Wf1 stacked twice (for row halves)
    x_bf1 = dram_in("bf1", [16, 1])
    x_wf2e = dram_in("wf2e", [17, 1])

    out_scores = nc.dram_tensor("scores", [18, 1], F32, kind="ExternalOutput")
    # clsv[:, 0] = sum(exp(shifted)); clsv[:, 1] = selected shifted logit.
    # Host computes ln(se) - sel, so the device never needs the Ln table.
    out_cls = nc.dram_tensor("clsv", [S, 2], F32, kind="ExternalOutput")

    with tile.TileContext(nc) as tc:
        with (
            tc.tile_pool(name="const", bufs=1) as cpool,
            tc.tile_pool(name="work", bufs=1) as wpool,
            tc.tile_pool(name="patch", bufs=1) as ppool,
            tc.tile_pool(name="hdd", bufs=4) as hpool,
            tc.tile_pool(name="gscr", bufs=3) as spool,
            tc.tile_pool(name="psmall", bufs=2, space="PSUM") as pc_pool,
            tc.tile_pool(name="pbig", bufs=3, space="PSUM") as pb_pool,
        ):
            # ---- inputs to SBUF; order matters: conv-critical first on sync ----
            def c_tile(src, shape, dtype=F32, eng=None):
                t = cpool.tile(list(shape), dtype, tag=src.name)
                (eng or nc.sync).dma_start(out=t[:], in_=src[:])
                return t

            patches_sb = ppool.tile([128, 2, 1024], BF16)
            nc.sync.dma_start(out=patches_sb[:], in_=x_patches[:])
            w1_sb = c_tile(x_w1, [128, 32], BF16)
            bc1_sb = c_tile(x_bc1, [128, 1])
            w2_sb = c_tile(x_w2, [128, 9 * 48], BF16)
            bc2_sb = c_tile(x_bc2, [128, 1])
            w3_sb = c_tile(x_w3, [128, 9 * 64], BF16)
            bc3_sb = c_tile(x_bc3, [64, 1])

            featc = wpool.tile([C2, S * M], BF16)
            # non-critical consts; optionally spread across the scalar queue
            # (only SP/Activation/gpsimd can initiate DMAs)
            alt = nc.scalar if KDMA == "spread" else nc.sync
            nc.sync.dma_start(out=featc[64:66, :], in_=x_coords[:])
            w1a_sb = c_tile(x_w1a, [C2, H1], BF16, eng=alt)
            w1b_sb = c_tile(x_w1b, [C2, H1], BF16, eng=alt)
            bg1_sb = c_tile(x_bg1, [H1, 1], eng=alt)
            wg2_sb = c_tile(x_wg2, [H1, CO], BF16, eng=alt)
            bg2_sb = c_tile(x_bg2, [2 * CO, 1], eng=alt)
            wle_sb = c_tile(x_wle, [65, NCls])
            onehot_sb = c_tile(x_onehot, [S, NCls])
            wf1d_sb = c_tile(x_wf1d, [2 * CO, 16])
            bf1_sb = c_tile(x_bf1, [16, 1])
            wf2e_sb = c_tile(x_wf2e, [17, 1])

            def r32(ap):
                return ap

            # ---- conv1: [27]->[32], 64x64 -> 32x32 (stride 2, im2col'd) ----
            # image j at partition base 32*(j%4), slot j//4 (both in and out),
            # so 3 matmuls run concurrently in distinct PE row+col groups.
            c1sb = wpool.tile([128, 2, 33, 33], BF16)
            c2sb = wpool.tile([128, 3, 17, 17], BF16)
            nc.gpsimd.memset(c1sb[:, :, 32, :], 0.0)
            nc.gpsimd.memset(c1sb[:, :, 0:32, 32], 0.0)
            nc.gpsimd.memset(c2sb[:, :, 16, :], 0.0)
            nc.gpsimd.memset(c2sb[:, :, 0:16, 16], 0.0)

            for h in range(2):
                for grp in ((0, 1, 2), (3, 4, 5)):
                    ps1 = pb_pool.tile([128, 16, 32], F32, tag="gps")
                    for img in grp:
                        pb = 32 * (img % 4)
                        slot = img // 4
                        nc.tensor.matmul(
                            ps1[pb:pb + 32].rearrange("p a b -> p (a b)"),
                            r32(w1_sb[pb:pb + 27, :]),
                            r32(patches_sb[pb:pb + 27, slot,
                                           h * 512:(h + 1) * 512]),
                            start=True, stop=True, tile_position=(pb, pb))
                    for img in grp:
                        pb = 32 * (img % 4)
                        slot = img // 4
                        # relu(x + bc1) -> padded layout
                        out_ap = c1sb[pb:pb + 32, slot,
                                      h * 16:(h + 1) * 16, 0:32]
                        if img % 3 == 2:
                            nc.scalar.activation(out_ap, ps1[pb:pb + 32],
                                                 AF.Relu,
                                                 bias=bc1_sb[pb:pb + 32])
                        else:
                            nc.vector.tensor_scalar(out_ap, ps1[pb:pb + 32],
                                                    bc1_sb[pb:pb + 32],
                                                    0.0, op0=OP.add, op1=OP.max)

            # ---- conv2: [32]->[48], 32x32 -> 16x16 ----
            # image j reads c1 at row base 32*(j%4), writes at col base
            # 64*(j%2), slot j//2; the two images of a pair run concurrently.
            for ip in range(3):      # image pairs
                tiles = []
                for img in (2 * ip, 2 * ip + 1):
                    rb, cb, slot = 32 * (img % 4), 64 * (img % 2), img // 4
                    ps2 = pb_pool.tile([128, 16, 16], F32, tag="gps")
                    for k, (dy, dx) in enumerate(
                            (dy, dx) for dy in range(3) for dx in range(3)):
                        nc.tensor.matmul(
                            ps2[cb:cb + 48],
                            r32(w2_sb[rb:rb + 32, k * 48:(k + 1) * 48]),
                            r32(c1sb[rb:rb + 32, slot,
                                     dy:dy + 31:2, dx:dx + 31:2]),
                            start=(k == 0), stop=(k == 8),
                            tile_position=(rb, cb))
                    tiles.append((img, cb, ps2))
                for img, cb, ps2 in tiles:
                    out_ap = c2sb[cb:cb + 48, img // 2, 0:16, 0:16]
                    if ip % 2 == 0:
                        nc.scalar.activation(out_ap, ps2[cb:cb + 48], AF.Relu,
                                             bias=bc2_sb[cb:cb + 48])
                    else:
                        nc.vector.tensor_scalar(out_ap, ps2[cb:cb + 48],
                                                bc2_sb[cb:cb + 48], 0.0,
                                                op0=OP.add, op1=OP.max)

            # ---- conv3: [48]->[64], 16x16 -> 8x8 ----
            # image j reads c2 at row base 64*(j%2); even/odd images run
            # concurrently into SEPARATE psum tiles (same output partitions,
            # so they must not share a bank: `start` clears per-bank bits).
            ps3e = pc_pool.tile([64, 3, D, D], F32, tag="psc")
            ps3o = pc_pool.tile([64, 3, D, D], F32, tag="psc")
            for img in range(S):
                rb = 64 * (img % 2)
                ps3 = ps3o if img % 2 else ps3e
                for k, (dy, dx) in enumerate(
                        (dy, dx) for dy in range(3) for dx in range(3)):
                    nc.tensor.matmul(
                        ps3[:, img // 2],
                        r32(w3_sb[rb:rb + 48, k * 64:(k + 1) * 64]),
                        r32(c2sb[rb:rb + 48, img // 2,
                                 dy:dy + 15:2, dx:dx + 15:2]),
                        start=(k == 0), stop=(k == 8),
                        tile_position=(rb, 0))
            fview = featc[0:64, :].rearrange("p (i m) -> p i m", m=M)
            nc.scalar.activation(fview[:, 0:6:2, :],
                                 ps3e[:].rearrange("p i a b -> p i (a b)"),
                                 AF.Relu, bias=bc3_sb[:])
            nc.scalar.activation(fview[:, 1:6:2, :],
                                 ps3o[:].rearrange("p i a b -> p i (a b)"),
                                 AF.Relu, bias=bc3_sb[:])

            # ---- u / v ----
            psu = pc_pool.tile([H1, S * M], F32, tag="psc")
            psv = pc_pool.tile([H1, S * M], F32, tag="psc")
            nc.tensor.matmul(psu[:], r32(w1a_sb[:]), r32(featc[:]), start=True, stop=True)
            nc.tensor.matmul(psv[:], r32(w1b_sb[:]), r32(featc[:]), start=True, stop=True)
            u_f32 = wpool.tile([H1, S * M], F32)
            v_bf = wpool.tile([H1, S * M], BF16)
            v_f32 = wpool.tile([H1, S * M], F32)
            nc.scalar.activation(u_f32[:], psu[:], AF.Copy)
            nc.vector.tensor_scalar(v_bf[:], psv[:], bg1_sb[:], None, op0=OP.add)
            nc.vector.tensor_scalar(v_f32[:], psv[:], bg1_sb[:], None, op0=OP.add)

            # ---- cls head (overlaps relation; DMA result out early) ----
            fme = wpool.tile([65, S], F32)
            nc.gpsimd.memset(fme[:], 1.0)
            nc.vector.tensor_reduce(
                fme[0:64, :], featc[0:64, :].rearrange("p (i m) -> p i m", m=M),
                axis=mybir.AxisListType.X, op=OP.add)
            psl = pc_pool.tile([S, NCls], F32, tag="psc")
            nc.tensor.matmul(psl[:], r32(fme[:]), r32(wle_sb[:]), start=True, stop=True)
            mx = wpool.tile([S, 1], F32)
            nc.vector.tensor_reduce(mx[:], psl[:], axis=mybir.AxisListType.X, op=OP.max)
            shifted = wpool.tile([S, NCls], F32)
            nc.vector.tensor_scalar(shifted[:], psl[:], mx[:], None, op0=OP.subtract)
            escr = wpool.tile([S, NCls], F32)
            se = wpool.tile([S, 1], F32)
            nc.scalar.activation(escr[:], shifted[:], AF.Exp, accum_out=se[:])
            selscr = wpool.tile([S, NCls], F32)
            sel = wpool.tile([S, 1], F32)
            nc.vector.tensor_tensor(selscr[:], shifted[:], onehot_sb[:], op=OP.mult)
            nc.vector.tensor_reduce(sel[:], selscr[:], axis=mybir.AxisListType.X, op=OP.add)
            nc.sync.dma_start(out=out_cls[:, 0:1], in_=se[:])
            nc.sync.dma_start(out=out_cls[:, 1:2], in_=sel[:])

            # ---- relation stage ----
            # xf_cols[:, (qh*9 + jl*3 + duo) + 18*gh] accumulates one evac op's
            # sum over (16q x 64p); combine gh then qh afterwards.
            xf_cols = wpool.tile([2 * CO, 36], F32)

            # engine assignment pattern for the 32 hdd-gen q ops of each unit
            gps_slots = set(int(i * 32 / KGPS) for i in range(KGPS)) if KGPS else set()
            rest = [s for s in range(32) if s not in gps_slots]
            act_slots = set(rest[int((i + 0.5) * len(rest) / KACT)]
                            for i in range(KACT)) if KACT else set()
            dve_evac_slots = (set(int((i + 0.5) * 6 / KDVE_EVAC)
                                  for i in range(KDVE_EVAC))
                              if KDVE_EVAC else set())

            for jl in range(3):
                for qh in range(2):
                    hdd = hpool.tile([H1, 32, S * M], BF16, tag="hdd")
                    for ql in range(32):
                        q = qh * 32 + ql
                        ucol = u_f32[:, jl * M + q: jl * M + q + 1]
                        if ql in act_slots:
                            nc.scalar.activation(hdd[:, ql, :], v_f32[:],
                                                 AF.Relu, bias=ucol)
                        elif ql in gps_slots:
                            nc.gpsimd.tensor_scalar(hdd[:, ql, :], v_f32[:],
                                                    ucol, 0.0,
                                                    op0=OP.add, op1=OP.max)
                        else:
                            nc.vector.tensor_scalar(hdd[:, ql, :], v_bf[:],
                                                    ucol, 0.0,
                                                    op0=OP.add, op1=OP.max)
                    for duo in range(3):
                        iA, iB = 2 * duo, 2 * duo + 1
                        for gh in range(2):
                            ps = pb_pool.tile([2 * CO, 1024], F32, tag="gps")
                            for q2 in range(2):
                                qg = gh * 2 + q2
                                nc.tensor.matmul(
                                    ps[0:CO, q2 * 512:(q2 + 1) * 512],
                                    wg2_sb[:],
                                    hdd[:, qg * 8:(qg + 1) * 8, iA * M:(iA + 1) * M],
                                    start=True, stop=True)
                                nc.tensor.matmul(
                                    ps[CO:2 * CO, q2 * 512:(q2 + 1) * 512],
                                    wg2_sb[:],
                                    hdd[:, qg * 8:(qg + 1) * 8, iB * M:(iB + 1) * M],
                                    start=True, stop=True,
                                    tile_position=(0, 64))
                            ucol_i = (qh * 9 + jl * 3 + duo) + 18 * gh
                            gscr = spool.tile([2 * CO, 1024], BF16, tag="gscr")
                            if (duo * 2 + gh) in dve_evac_slots:
                                nc.vector._custom_dve(
                                    RELU_BIAS_SUM, out=gscr[:], in0=ps[:],
                                    s0=bg2_sb[:],
                                    accum_out=xf_cols[:, ucol_i:ucol_i + 1])
                            else:
                                nc.scalar.activation(
                                    gscr[:], ps[:], AF.Relu,
                                    bias=bg2_sb[:],
                                    accum_out=xf_cols[:, ucol_i:ucol_i + 1])

            # ---- score head ----
            # sum the two gh-halves, then the two qh-halves
            xf18 = wpool.tile([2 * CO, 18], F32)
            nc.vector.tensor_tensor(
                xf18[:], xf_cols[:, 0:18], xf_cols[:, 18:36], op=OP.add)
            xf_pair = wpool.tile([2 * CO, 9], F32)
            nc.vector.tensor_tensor(
                xf_pair[:], xf18[:, 0:9], xf18[:, 9:18], op=OP.add)
            # assemble xf for both partition halves into one base-0 tile:
            # even i (rows 0:64) -> cols 0:9, odd i -> cols 9:18 (SBUF DMA)
            xf_ext = wpool.tile([CO, 18], F32)
            nc.vector.tensor_copy(xf_ext[:, 0:9], xf_pair[0:CO, :])
            nc.sync.dma_start(out=xf_ext[:, 9:18], in_=xf_pair[CO:2 * CO, :])
            psh1 = pc_pool.tile([16, 18], F32, tag="psc")
            nc.tensor.matmul(psh1[:], r32(wf1d_sb[0:CO, :]),
                             r32(xf_ext[:]), start=True, stop=True)
            h1e = wpool.tile([17, 18], F32)
            nc.gpsimd.memset(h1e[:], 1.0)
            nc.scalar.activation(h1e[0:16, :], psh1[:], AF.Relu, bias=bf1_sb[:])
            psh2 = pc_pool.tile([18, 1], F32, tag="psc")
            nc.tensor.matmul(psh2[:], r32(h1e[:]), r32(wf2e_sb[:]),
                             start=True, stop=True)
            en = wpool.tile([18, 1], F32)
            nc.scalar.activation(en[:], psh2[:], AF.Exp, scale=-1.0)
            ep1 = wpool.tile([18, 1], F32)
            nc.vector.tensor_scalar(ep1[:], en[:], 1.0, None, op0=OP.add)
            sc = wpool.tile([18, 1], F32)
            nc.vector.reciprocal(sc[:], ep1[:])
            nc.sync.dma_start(out=out_scores[:], in_=sc[:])
    nc.compile()
    return nc


_NC_CACHE = None


def _get_nc():
    global _NC_CACHE
    if _NC_CACHE is None:
        _NC_CACHE = _build_nc()
    return _NC_CACHE


def _host_prep(inputs):
    ins = {k: np.asarray(v) for k, v in inputs.items()}
    x = np.concatenate([ins['support_x'], ins['query_x']], axis=1)
    lab = np.concatenate([ins['support_y'], ins['query_y']], axis=1)

    xpad = np.pad(x.astype(np.float32), ((0, 0), (0, 0), (0, 0), (0, 1), (0, 1)))
    win = np.lib.stride_tricks.sliding_window_view(xpad, (3, 3), axis=(3, 4))
    win = win[:, :, :, ::2, ::2]
    patches = win.transpose(0, 2, 5, 6, 1, 3, 4).reshape(B, 27, S, 1024)
    patches = np.ascontiguousarray(patches, np.float32)

    f32 = np.float32
    bf16 = ml_dtypes.bfloat16
    w1s = np.ascontiguousarray(ins['k1'].reshape(32, 27).T, f32)
    w1 = np.zeros((128, 32), f32)
    for k in range(4):
        w1[32 * k:32 * k + 27, :] = w1s
    w1 = w1.astype(bf16)
    w2s = np.ascontiguousarray(ins['k2'].transpose(1, 2, 3, 0).reshape(32, 9 * 48), f32)
    w2 = np.zeros((128, 9 * 48), f32)
    for k in range(4):
        w2[32 * k:32 * k + 32, :] = w2s
    w2 = w2.astype(bf16)
    w3s = np.ascontiguousarray(ins['k3'].transpose(1, 2, 3, 0).reshape(48, 9 * 64), f32)
    w3 = np.zeros((128, 9 * 64), f32)
    for k in range(2):
        w3[64 * k:64 * k + 48, :] = w3s
    w3 = w3.astype(bf16)

    ii = np.arange(D, dtype=f32) / D
    coord = np.stack([np.broadcast_to(ii[:, None], (D, D)),
                      np.broadcast_to(ii[None, :], (D, D))]).reshape(2, M)
    coords = np.ascontiguousarray(np.tile(coord, (1, S)), f32).astype(bf16)

    onehots = np.zeros((B, S, NCls), f32)
    for b in range(B):
        onehots[b, np.arange(S), lab[b]] = 1.0

    Wg1 = ins['Wg1'].astype(f32)
    common = dict(
        w1=w1, w2=w2, w3=w3,
        bc1=np.ascontiguousarray(
            np.vstack([ins['bc1'].reshape(32, 1)] * 4), f32),
        bc2=np.ascontiguousarray(np.vstack(
            [ins['bc2'].reshape(48, 1), np.zeros((16, 1)),
             ins['bc2'].reshape(48, 1), np.zeros((16, 1))]), f32),
        bc3=np.ascontiguousarray(ins['bc3'].reshape(64, 1), f32),
        coords=coords,
        wle=np.ascontiguousarray(
            np.vstack([ins['Wlog'].astype(f32) / M, ins['blog'][None, :].astype(f32)])),
        w1a=np.ascontiguousarray(Wg1[:C2]).astype(bf16),
        w1b=np.ascontiguousarray(Wg1[C2:]).astype(bf16),
        bg1=np.ascontiguousarray(ins['bg1'].reshape(H1, 1), f32),
        wg2=np.ascontiguousarray(ins['Wg2'], f32).astype(ml_dtypes.bfloat16),
        bg2_2=np.ascontiguousarray(np.tile(ins['bg2'].astype(f32), 2).reshape(2 * CO, 1)),
        wf1d=np.ascontiguousarray(
            np.vstack([ins['Wf1'].astype(f32), ins['Wf1'].astype(f32)])),
        bf1=np.ascontiguousarray(ins['bf1'].reshape(16, 1), f32),
        wf2e=np.ascontiguousarray(
            np.vstack([ins['Wf2'].astype(f32), ins['bf2'].reshape(1, 1).astype(f32)])),
    )
    in_maps = []
    for core in range(N_CORES):
        b, half = core // 2, core % 2
        # odd cores see images in rotated order so the program's local
        # j in {0,1,2} maps to global j in {3,4,5}
        perm = (0, 1, 2, 3, 4, 5) if half == 0 else (3, 4, 5, 0, 1, 2)
        m = dict(common)
        p4 = np.zeros((128, 2, 1024), f32)
        for j in range(S):
            p4[32 * (j % 4):32 * (j % 4) + 27, j // 4, :] = patches[b][:, perm[j], :]
        m['patches'] = p4.astype(ml_dtypes.bfloat16)
        m['onehot'] = np.ascontiguousarray(onehots[b][list(perm)])
        in_maps.append(m)
    return in_maps, lab


def _host_post(results, lab):
    P = np.zeros((B, S, S), np.float32)
    cls_terms = np.zeros((B, S), np.float32)
    for core in range(N_CORES):
        b, half = core // 2, core % 2
        perm = (0, 1, 2, 3, 4, 5) if half == 0 else (3, 4, 5, 0, 1, 2)
        sc = results[core]["scores"].reshape(18)
        # score col layout: k < 9: (jl = k//3, duo = k%3, i = 2*duo);
        #                   k >= 9: i = 2*duo + 1
        for k in range(18):
            kk = k % 9
            jl, duo = kk // 3, kk % 3
            i = 2 * duo + (1 if k >= 9 else 0)
            P[b, perm[i], perm[jl]] = sc[k]
        if half == 0:
            sesel = results[core]["clsv"].reshape(S, 2)
            cls_terms[b] = np.log(sesel[:, 0]) - sesel[:, 1]
    cls_loss = np.float32(cls_terms.mean())
    y = (lab[:, :, None] == lab[:, None, :]).astype(np.float32)
    Pt = P.transpose(0, 2, 1)
    sym, anti = np.float32(0.5) * (P + Pt), np.float32(0.5) * (P - Pt)
    sym_n = np.sqrt((sym ** 2).sum(axis=(1, 2)))
    anti_n = np.sqrt((anti ** 2).sum(axis=(1, 2)))
    sym_loss = np.float32(((sym_n - anti_n) / (sym_n + anti_n)).mean())
    euc_loss = np.float32(((P - y) ** 2).mean())
    rn_loss = np.float32(euc_loss - np.float32(0.1) * sym_loss)
    return np.float32(cls_loss), np.float32(rn_loss), np.float32(sym_loss)


def run_spmd(inputs, trace=False, **kwargs):
    nc = _get_nc()
    in_maps, lab = _host_prep(inputs)
    res = run_bass_kernel_spmd(nc, in_maps, list(range(N_CORES)),
                               trace=trace, **kwargs)
    return _host_post(res.results, lab), res


def kernel(**inputs):
    out, _ = run_spmd(inputs)
    return out
